# revision 2
# baseline (speedup 1.0000x reference)
"""Bass/Trainium2 kernel v3 for nn_BboxIoULoss (topk_masking).

loss = sum((1 - diou) * mask) / sum(mask),  mask = (iou1ds > 0.5) OR top-3
(top-3 subsumed by the threshold on these inputs; checked, numpy fallback
otherwise).

Strategy (8 cores data-parallel over M; per core 128 target partitions):
  - all four input slabs (iou, s1, e1, c1=s1+e1) ship as fp8 e4m3
    (4.2 MB/core): B1/B2 enter the loss scaled by ~0.019 so %-level error
    there moves the answer ~1e-3 << 2e-2 tolerance.
  - ACT: mask = Sigmoid(4096*iou - 2048) (hard 0/1 step at the fp8 grid
    point 0.5; values == 0.5 get weight 1/2, statistically neutral) with
    fused row-accumulate -> A; cd = c1 - c2 via Identity + per-partition
    bias.  A tiny warm-up activation hoists ACT_TABLE_LOAD to t~0.
  - DVE: two runtime-registered custom fused ops
      RW4_ANT:   rw  = relu(min(e1,e2) - max(s1,s2))        [4 stages]
      RENC8_ANT: renc ~= 1/(max(e1,e2) - min(s1,s2))        [8 stages]
        (enclose + bitwise-NOT reciprocal seed + 1 Newton step, ~0.4%)
    plus stock 2x-mode rm = mask*renc and qm = cd*rm.
  - TensorE: block-diagonal gram accumulation
      PB1 += rw_blk^T @ rm_blk   (trace = B1 = sum mask*iou)
      PB2 += qm_blk^T @ qm_blk   (trace = 4*B2 = 4*sum mask*pen)
  - answer = (A - B1 + B2) / A   (host sums traces in float64)
"""

import os
import ml_dtypes
import numpy as np

import concourse.bass as bass
import concourse.tile as tile
import concourse.mybir as mybir
from concourse import bacc, bass_utils

F32 = mybir.dt.float32
BF16 = mybir.dt.bfloat16
F8 = mybir.dt.float8e4
AF = mybir.ActivationFunctionType
OP = mybir.AluOpType

S = 64
T = 16
N = 128
M = S * T                  # 1024
P = N * (N + 1) // 2       # 8256
TOPK = 3
NCORES = 8
ML = M // NCORES           # 128 targets / core (= partitions)
W = S // NCORES            # 8 samples / core
NCH = int(os.environ.get("BBK2_NCH", "6"))
CH = P // NCH              # free-dim chunk
BLK = 128                  # matmul block width


def _patch_act_tables():
    """Pin one activation table-set (sigmoid/identity/copy) so the
    scheduler emits a single ACT_TABLE_LOAD."""
    import functools
    import concourse.hw_specs as _hw

    orig = _hw.get_activation_tables.__wrapped__

    def only_sigmoid(arch):
        tabs = orig(arch)
        name = "sigmoid_and_others"
        if name not in tabs:
            return tabs
        return {k: (v if k == name else set()) for k, v in tabs.items()}

    _hw.get_activation_tables = functools.cache(only_sigmoid)
    bacc.get_activation_tables = _hw.get_activation_tables


_OPS_REGISTERED = {}


def _register_dve_ops():
    """Register the two fused DVE ops at runtime (dve_ops.py is read-only)."""
    if _OPS_REGISTERED:
        return _OPS_REGISTERED
    import concourse.dve_ops as dve_ops
    from concourse.dve_spec import (
        Spec, Src0, Src1, C0, C1, C2, One, maxx, minn, lower, _has_src1,
        Bin, AluOp,
    )
    from concourse.dve_uop import (
        DveOpSpec, UopConfig, UopDpConfig, Trigger, InpSel, OutSel, OutPath,
        AluInp, DelayInp, ENABLE, AluOp as UAluOp,
    )
    from concourse.dve_table_gen import dve_ver_for

    ver = dve_ver_for("TRN2")

    def _rw4_2x_uop():
        """Hand-packed 2x_1P program for RW4: lo chain on blocks 0-3
        (result rides delay lane 0 to the end), hi chain on blocks 4-7
        (result in block 7's ALU flop).  Mirrors the stock tensor_tensor
        2x slot conventions (SRC_*_HI input lanes, write0_en_lo+hi)."""
        u = UopConfig()
        for lane, s in [
            (1, InpSel.SRC_1), (2, InpSel.CONST_1), (3, InpSel.SRC_0),
            (4, InpSel.CONST_0), (5, InpSel.SRC_0_HI), (6, InpSel.SRC_1_HI),
        ]:
            u.enable_input(s, lane)
        u.require_inp0 = ENABLE
        u.require_inp1 = ENABLE
        u.trigger = (Trigger.SRC_TENSOR_DONE, Trigger.NONE, Trigger.NONE)
        B = u.datapath_config
        # at block 0: PREV_DELAY_n = input lane n+1:
        #   D0=b_lo D1=e2 D2=a_lo D3=s2 D4=a_hi D5=b_hi
        B[0].enable_alu(UAluOp.MIN, AluInp.PREV_DELAY_0, AluInp.PREV_DELAY_1)
        B[0].pass_through_delay(1, 2, 3, 4, 5)          # v_lo in flop
        B[1].enable_alu(UAluOp.MAX, AluInp.PREV_DELAY_2, AluInp.PREV_DELAY_3)
        B[1].enable_delay_from_src(DelayInp.PREV_ALU_OUT, 0)   # D0 <- v_lo
        B[1].pass_through_delay(1, 3, 4, 5)             # u_lo in flop
        B[2].enable_alu(UAluOp.MAX, AluInp.PREV_DELAY_0, AluInp.PREV_ALU_OUT)
        B[2].enable_delay_from_src(DelayInp.PREV_ALU_OUT, 0)   # D0 <- u_lo
        B[2].pass_through_delay(1, 3, 4, 5)             # m_lo in flop
        B[3].enable_alu(UAluOp.SUBTRACT, AluInp.PREV_ALU_OUT, AluInp.PREV_DELAY_0)
        B[3].pass_through_delay(1, 3, 4, 5)             # rw_lo in flop
        B[4].enable_alu(UAluOp.MAX, AluInp.PREV_DELAY_4, AluInp.PREV_DELAY_3)
        B[4].enable_delay_from_src(DelayInp.PREV_ALU_OUT, 0)   # D0 <- rw_lo
        B[4].pass_through_delay(1, 5)                   # u_hi in flop
        B[5].enable_alu(UAluOp.MIN, AluInp.PREV_DELAY_5, AluInp.PREV_DELAY_1)
        B[5].enable_delay_from_src(DelayInp.PREV_ALU_OUT, 2)   # D2 <- u_hi
        B[5].pass_through_delay(0)                      # v_hi in flop
        B[6].enable_alu(UAluOp.MAX, AluInp.PREV_ALU_OUT, AluInp.PREV_DELAY_2)
        B[6].pass_through_delay(0, 2)                   # m_hi in flop
        B[7].enable_alu(UAluOp.SUBTRACT, AluInp.PREV_ALU_OUT, AluInp.PREV_DELAY_2)
        B[7].pass_through_delay(0)                      # rw_hi in flop
        u.enable_output(OutSel.DELAY_0, OutPath.WR0_LO)   # rw_lo
        u.enable_output(OutSel.ALU_OUT, OutPath.WR0_HI)   # rw_hi
        return u

    def _reg(name, spec):
        row = dve_ops._CUSTOM_DVE_ROW_BASE + len(dve_ops.OPS)
        lowered = DveOpSpec(
            name=name, opcode=row, uops=lower(spec, ver=ver),
            rd1_en=_has_src1(spec),
        )
        op = dve_ops.DveOp(
            name, spec, subdim=False, uops_sha={ver: lowered.sha(ver)}
        )
        dve_ops.OPS.append(op)
        dve_ops.CUSTOM_DVE_SPECS[name] = spec
        dve_ops._SUB_OPCODE_FOR_NAME[name] = row
        return op

    # rw = relu(min(b, e2) - max(a, s2)) = max(v, u) - u
    _u = maxx(Src0, C0)
    _v = minn(Src1, C1)
    _rw_body = maxx(_v, _u) - _u

    def _rw_ref(in0, in1, s0, s1, imm2):
        u = np.maximum(in0.astype(np.float32), s0)
        v = np.minimum(in1.astype(np.float32), s1)
        return np.maximum(v, u) - u

    rw4_spec = Spec(body=_rw_body, reference=_rw_ref)
    if os.environ.get("BBK2_RW2X", "1") == "1":
        class _Rw4Op(dve_ops.DveOp):
            def compile(self, v):
                return DveOpSpec(
                    name=self.name,
                    opcode=dve_ops.get_dve_sub_opcode(self.name),
                    uops=lower(self.spec, ver=v),
                    rd1_en=_has_src1(self.spec),
                    uops_2x=[_rw4_2x_uop()],
                    perf_max=1,
                )

        row = dve_ops._CUSTOM_DVE_ROW_BASE + len(dve_ops.OPS)
        rw4 = _Rw4Op("RW4_ANT", rw4_spec, subdim=False, uops_sha={})
        dve_ops.OPS.append(rw4)
        dve_ops.CUSTOM_DVE_SPECS["RW4_ANT"] = rw4_spec
        dve_ops._SUB_OPCODE_FOR_NAME["RW4_ANT"] = row
    else:
        rw4 = _reg("RW4_ANT", rw4_spec)

    # renc ~= 1/enc, enc = max(b, e2) - min(a, s2):
    # seed y0 = bitcast(~enc) * c  lands enc*y0 in [16/17, 18/17] for
    # c = -4/17; one Newton step y1 = y0*(2 - enc*y0) -> |rel err| <= 0.35%
    _mn = minn(Src0, C0)
    _mx = maxx(Src1, C1)
    _enc = _mx - _mn
    _nx = Bin(AluOp.BITWISE_NOT, _enc, _enc)
    _y0 = _nx * C2
    _renc_body = _y0 * ((One + One) - _enc * _y0)

    def _renc_ref(in0, in1, s0, s1, imm2):
        mn = np.minimum(in0.astype(np.float32), s0)
        mx = np.maximum(in1.astype(np.float32), s1)
        enc = (mx - mn).astype(np.float32)
        nx = (~enc.view(np.int32)).view(np.float32)
        y0 = nx * imm2
        return y0 * (2.0 - enc * y0)

    renc8 = _reg("RENC8_ANT", Spec(body=_renc_body, reference=_renc_ref))

    _OPS_REGISTERED.update({"rw4": rw4, "renc8": renc8})
    return _OPS_REGISTERED


def _build_program():
    if not os.environ.get("BBK2_NOPATCH"):
        _patch_act_tables()
    ops = _register_dve_ops()
    rw4, renc8 = ops["rw4"], ops["renc8"]

    nc = bacc.Bacc(
        "TRN2", target_bir_lowering=False, debug=False, enable_asserts=False
    )
    # chunk-major slabs: rows [c*ML, (c+1)*ML) = chunk c
    iou_d = nc.dram_tensor("iou", [NCH * ML, CH], F8, kind="ExternalInput")
    ab_d = nc.dram_tensor("ab", [NCH * ML, 2 * CH], BF16, kind="ExternalInput")
    cm_d = nc.dram_tensor("cm", [NCH * ML, CH], F8, kind="ExternalInput")
    tgt_d = nc.dram_tensor("tgt", [ML, 4], F32, kind="ExternalInput")
    acc_d = nc.dram_tensor("acc", [ML, NCH], F32, kind="ExternalOutput")
    mm_d = nc.dram_tensor("mm", [ML, 2 * BLK], F32, kind="ExternalOutput")
    mmb_d = nc.dram_tensor("mmb", [ML, 2 * BLK], F32, kind="ExternalOutput")

    linearize = bool(int(os.environ.get("BBK2_LINEARIZE", "0")))
    with tile.TileContext(nc, linearize=linearize) as tc:
        with (
            tc.tile_pool(name="const", bufs=1) as cp,
            tc.tile_pool(name="inp", bufs=int(os.environ.get("BBK2_IBUFS", "4"))) as ip,
            tc.tile_pool(name="work", bufs=int(os.environ.get("BBK2_WBUFS", "4"))) as wp,
            tc.psum_pool(name="ps", bufs=1) as pp,
        ):
            tgt = cp.tile([ML, 4], F32)
            s2 = tgt[:, 0:1]
            e2 = tgt[:, 1:2]
            nc2 = tgt[:, 2:3]          # -c2 (ACT Identity bias)
            sgb = tgt[:, 3:4]          # -2048 (sigmoid step bias)

            acc = cp.tile([ML, NCH], F32)
            pb1 = pp.tile([ML, BLK], F32)
            pb2 = pp.tile([ML, BLK], F32)
            pb1b = pp.tile([ML, BLK], F32)
            pb2b = pp.tile([ML, BLK], F32)

            # dummy activation: hoists ACT_TABLE_LOAD to t~0, concurrent
            # with the first input DMAs
            warm = cp.tile([ML, 1], F32)
            nc.vector.memset(warm[:], 0.0)
            nc.scalar.activation(warm[:], warm[:], AF.Sigmoid, bias=0.0, scale=1.0)

            nc.sync.dma_start(tgt[:], tgt_d.ap())

            nblk = (CH + BLK - 1) // BLK
            for c in range(NCH):
                iot = ip.tile([ML, CH], F8, tag="iot")
                nc.sync.dma_start(iot[:], iou_d.ap()[c * ML : (c + 1) * ML, :])
                abt = ip.tile([ML, 2 * CH], BF16, tag="abt")
                nc.sync.dma_start(abt[:], ab_d.ap()[c * ML : (c + 1) * ML, :])
                ct = ip.tile([ML, CH], F8, tag="ct")
                nc.sync.dma_start(ct[:], cm_d.ap()[c * ML : (c + 1) * ML, :])
                a = abt[:, 0:CH]           # s1
                b = abt[:, CH : 2 * CH]    # e1
                c1 = ct[:]

                # mask = hard step(iou > 0.5), fused row accumulate -> A
                mask = wp.tile([ML, CH], BF16, tag="mask")
                nc.scalar.activation(
                    mask[:], iot[:], AF.Sigmoid,
                    bias=sgb, scale=4096.0,
                    accum_out=acc[:, c : c + 1],
                )

                rw = wp.tile([ML, CH], BF16, tag="rw")
                _bi = nc.vector._custom_dve(
                    rw4, out=rw[:], in0=a, in1=b, s0=s2, s1=e2
                )
                for _attr in ("ins", "instruction", "inst"):
                    _obj = getattr(_bi, _attr, None)
                    if _obj is not None and hasattr(_obj, "perf_max"):
                        _obj.perf_max = 1
                        break
                else:
                    if hasattr(_bi, "perf_max"):
                        _bi.perf_max = 1
                renc = wp.tile([ML, CH], BF16, tag="renc")
                nc.vector._custom_dve(
                    renc8, out=renc[:], in0=a, in1=b, s0=s2, s1=e2,
                    imm2=-4.0 / 17.0,
                )

                rm = wp.tile([ML, CH], BF16, tag="rm")
                nc.vector.tensor_tensor(rm[:], mask[:], renc[:], OP.mult)

                cd = wp.tile([ML, CH], BF16, tag="cd")
                nc.scalar.activation(
                    cd[:], c1, AF.Identity, bias=nc2, scale=1.0
                )

                qm = wp.tile([ML, CH], BF16, tag="qm")
                nc.vector.tensor_tensor(qm[:], cd[:], rm[:], OP.mult)

                # chunks 0..NCH-2 accumulate into pb1/pb2; the last chunk
                # into pb1b/pb2b so the big accumulators drain (copy + DMA)
                # while the last chunk computes
                t1 = pb1 if c < NCH - 1 else pb1b
                t2 = pb2 if c < NCH - 1 else pb2b
                for bi in range(nblk):
                    lo = bi * BLK
                    hi = min(CH, lo + BLK)
                    wb = hi - lo
                    first = c in (0, NCH - 1) and bi == 0
                    last = c in (NCH - 2, NCH - 1) and bi == nblk - 1
                    nc.tensor.matmul(
                        t1[0:wb, 0:wb], rw[:, lo:hi], rm[:, lo:hi],
                        start=first, stop=last,
                    )
                    nc.tensor.matmul(
                        t2[0:wb, 0:wb], qm[:, lo:hi], qm[:, lo:hi],
                        start=first, stop=last,
                    )
                if c == NCH - 2:
                    mma = cp.tile([ML, 2 * BLK], F32)
                    nc.scalar.copy(mma[:, 0:BLK], pb1[:])
                    nc.scalar.copy(mma[:, BLK : 2 * BLK], pb2[:])
                    nc.sync.dma_start(mm_d.ap(), mma[:])

            mmb = cp.tile([ML, 2 * BLK], F32)
            nc.scalar.copy(mmb[:, 0:BLK], pb1b[:])
            nc.scalar.copy(mmb[:, BLK : 2 * BLK], pb2b[:])
            nc.sync.dma_start(mmb_d.ap(), mmb[:])
            nc.sync.dma_start(acc_d.ap(), acc[:])

    nc.compile()
    return nc


_NC_CACHE = None


def _get_program():
    global _NC_CACHE
    if _NC_CACHE is None:
        _NC_CACHE = _build_program()
    return _NC_CACHE


def _reference_numpy(out_moments, tgt_moments, num_targets, iou2ds, mask2d):
    """Exact numpy replica of the jax reference (fallback path)."""
    M_, N_, _ = iou2ds.shape
    S_, P_, _ = out_moments.shape
    scatter = np.repeat(np.arange(S_), num_targets)
    om = out_moments[scatter].astype(np.float32)      # [M, P, 2]
    tg = tgt_moments[:, None, :].astype(np.float32)
    s1, e1 = om[..., 0], om[..., 1]
    s2, e2 = tg[..., 0], tg[..., 1]
    inter = np.clip(np.minimum(e1, e2) - np.maximum(s1, s2), 0.0, None)
    union = (e1 - s1) + (e2 - s2) - inter
    iou = inter / union
    enclose = np.maximum(e1, e2) - np.minimum(s1, s2)
    cdist = (s1 + e1) * 0.5 - (s2 + e2) * 0.5
    bbox_diou = iou - (cdist * cdist) / (enclose * enclose)
    flat_idx = np.nonzero(mask2d.reshape(-1))[0]
    iou1 = iou2ds.reshape(M_, -1)[:, flat_idx]
    kth = np.argpartition(-iou1, TOPK - 1, axis=1)[:, :TOPK]
    target_mask = np.zeros((M_, P_), np.float32)
    target_mask[np.arange(M_)[:, None], kth] = 1.0
    target_mask = np.where(iou1 > 0.5, 1.0, target_mask)
    loss = 1.0 - bbox_diou
    return np.float32((loss * target_mask).sum() / target_mask.sum())


def kernel(out_moments, tgt_moments, num_targets, iou2ds, mask2d):
    out_moments = np.asarray(out_moments, np.float32)
    tgt_moments = np.asarray(tgt_moments, np.float32)
    num_targets = np.asarray(num_targets, np.int32)
    iou2ds = np.asarray(iou2ds, np.float32)
    mask2d_np = np.asarray(mask2d)

    uniform = bool(np.all(num_targets == T))
    triu_ok = bool(
        np.array_equal(mask2d_np, np.triu(np.ones((N, N), dtype=bool)))
    )
    if not (uniform and triu_ok and iou2ds.shape == (M, N, N)):
        return _reference_numpy(
            out_moments, tgt_moments, num_targets, iou2ds, mask2d_np
        )

    nc = _get_program()
    f8 = ml_dtypes.float8_e4m3

    # host layout prep: triu-compact iou2ds to p-order, fp8
    flat_idx = np.nonzero(mask2d_np.reshape(-1))[0]
    iou1 = iou2ds.reshape(M, -1)[:, flat_idx].astype(f8)     # [M, P]
    s1 = out_moments[..., 0]                                  # [S, P] f32
    e1 = out_moments[..., 1]
    bf16 = ml_dtypes.bfloat16
    c1 = (s1 + e1).astype(f8)
    s1 = s1.astype(bf16)
    e1 = e1.astype(bf16)

    in_maps = []
    for k in range(NCORES):
        sl_m = slice(k * ML, (k + 1) * ML)
        sl_s = slice(k * W, (k + 1) * W)
        # replicate each sample's moments across its 16 target partitions
        s1k = np.repeat(s1[sl_s], T, axis=0)                  # [128, P]
        e1k = np.repeat(e1[sl_s], T, axis=0)
        c1k = np.repeat(c1[sl_s], T, axis=0)
        iouk = iou1[sl_m]
        pio = np.empty((NCH, ML, CH), f8)
        pab = np.empty((NCH, ML, 2 * CH), bf16)
        pcm = np.empty((NCH, ML, CH), f8)
        for c in range(NCH):
            sl_p = slice(c * CH, (c + 1) * CH)
            pio[c] = iouk[:, sl_p]
            pab[c, :, 0:CH] = s1k[:, sl_p]
            pab[c, :, CH : 2 * CH] = e1k[:, sl_p]
            pcm[c] = c1k[:, sl_p]
        tgtk = tgt_moments[sl_m]                              # [128, 2] f32
        tgt4 = np.zeros((ML, 4), np.float32)
        tgt4[:, 0] = tgtk[:, 0]
        tgt4[:, 1] = tgtk[:, 1]
        tgt4[:, 2] = -(tgtk[:, 0] + tgtk[:, 1])               # -c2
        tgt4[:, 3] = -2048.0                                  # sigmoid bias
        in_maps.append(
            {
                "iou": np.ascontiguousarray(pio.reshape(NCH * ML, CH)),
                "ab": np.ascontiguousarray(pab.reshape(NCH * ML, 2 * CH)),
                "cm": np.ascontiguousarray(pcm.reshape(NCH * ML, CH)),
                "tgt": tgt4,
            }
        )

    trace = bool(int(os.environ.get("BBK_TRACE", "0")))
    res = bass_utils.run_bass_kernel_spmd(
        nc, in_maps, core_ids=list(range(NCORES)), trace=trace
    )
    if trace:
        kernel.last_exec_time_ns = res.exec_time_ns

    acc = np.stack([res.results[k]["acc"] for k in range(NCORES)])  # [8,128,NCH]
    mm = np.stack(
        [
            res.results[k]["mm"].astype(np.float64)
            + res.results[k]["mmb"].astype(np.float64)
            for k in range(NCORES)
        ]
    )
    acc64 = acc.astype(np.float64)
    a_rows = acc64.sum(axis=2)                     # per-core per-row counts
    A = a_rows.sum()
    mm64 = mm.astype(np.float64)
    B1 = np.trace(mm64[:, :, 0:BLK], axis1=1, axis2=2).sum()
    B2 = np.trace(mm64[:, :, BLK : 2 * BLK], axis1=1, axis2=2).sum() / 4.0

    if a_rows.min() < 4 * TOPK:
        # threshold may not subsume top-3: replicate reference on host
        return _reference_numpy(
            out_moments, tgt_moments, num_targets, iou2ds, mask2d_np
        )

    return np.float32((A - B1 + B2) / A)


# revision 3
# speedup vs baseline: 1.0727x; 1.0727x over previous
"""Bass/Trainium2 kernel v3 for nn_BboxIoULoss (topk_masking).

loss = sum((1 - diou) * mask) / sum(mask),  mask = (iou1ds > 0.5) OR top-3
(top-3 subsumed by the threshold on these inputs; checked, numpy fallback
otherwise).

Strategy (8 cores data-parallel over M; per core 128 target partitions):
  - all four input slabs (iou, s1, e1, c1=s1+e1) ship as fp8 e4m3
    (4.2 MB/core): B1/B2 enter the loss scaled by ~0.019 so %-level error
    there moves the answer ~1e-3 << 2e-2 tolerance.
  - ACT: mask = Sigmoid(4096*iou - 2048) (hard 0/1 step at the fp8 grid
    point 0.5; values == 0.5 get weight 1/2, statistically neutral) with
    fused row-accumulate -> A; cd = c1 - c2 via Identity + per-partition
    bias.  A tiny warm-up activation hoists ACT_TABLE_LOAD to t~0.
  - DVE: two runtime-registered custom fused ops
      RW4_ANT:   rw  = relu(min(e1,e2) - max(s1,s2))        [4 stages]
      RENC8_ANT: renc ~= 1/(max(e1,e2) - min(s1,s2))        [8 stages]
        (enclose + bitwise-NOT reciprocal seed + 1 Newton step, ~0.4%)
    plus stock 2x-mode rm = mask*renc and qm = cd*rm.
  - TensorE: block-diagonal gram accumulation
      PB1 += rw_blk^T @ rm_blk   (trace = B1 = sum mask*iou)
      PB2 += qm_blk^T @ qm_blk   (trace = 4*B2 = 4*sum mask*pen)
  - answer = (A - B1 + B2) / A   (host sums traces in float64)
"""

import os
import ml_dtypes
import numpy as np

import concourse.bass as bass
import concourse.tile as tile
import concourse.mybir as mybir
from concourse import bacc, bass_utils

F32 = mybir.dt.float32
BF16 = mybir.dt.bfloat16
F8 = mybir.dt.float8e4
AF = mybir.ActivationFunctionType
OP = mybir.AluOpType

S = 64
T = 16
N = 128
M = S * T                  # 1024
P = N * (N + 1) // 2       # 8256
TOPK = 3
NCORES = 8
ML = M // NCORES           # 128 targets / core (= partitions)
W = S // NCORES            # 8 samples / core
NCH = int(os.environ.get("BBK2_NCH", "6"))
CH = P // NCH              # free-dim chunk
BLK = 128                  # matmul block width


def _patch_act_tables():
    """Pin one activation table-set (sigmoid/identity/copy) so the
    scheduler emits a single ACT_TABLE_LOAD."""
    import functools
    import concourse.hw_specs as _hw

    orig = _hw.get_activation_tables.__wrapped__

    def only_sigmoid(arch):
        tabs = orig(arch)
        name = "sigmoid_and_others"
        if name not in tabs:
            return tabs
        return {k: (v if k == name else set()) for k, v in tabs.items()}

    _hw.get_activation_tables = functools.cache(only_sigmoid)
    bacc.get_activation_tables = _hw.get_activation_tables


_OPS_REGISTERED = {}


def _register_dve_ops():
    """Register the two fused DVE ops at runtime (dve_ops.py is read-only)."""
    if _OPS_REGISTERED:
        return _OPS_REGISTERED
    import concourse.dve_ops as dve_ops
    from concourse.dve_spec import (
        Spec, Src0, Src1, C0, C1, C2, One, maxx, minn, lower, _has_src1,
        Bin, AluOp,
    )
    from concourse.dve_uop import (
        DveOpSpec, UopConfig, UopDpConfig, Trigger, InpSel, OutSel, OutPath,
        AluInp, DelayInp, ENABLE, AluOp as UAluOp,
    )
    from concourse.dve_table_gen import dve_ver_for

    ver = dve_ver_for("TRN2")

    def _rw4_2x_uop():
        """Hand-packed 2x_1P program for RW4: lo chain on blocks 0-3
        (result rides delay lane 0 to the end), hi chain on blocks 4-7
        (result in block 7's ALU flop).  Mirrors the stock tensor_tensor
        2x slot conventions (SRC_*_HI input lanes, write0_en_lo+hi)."""
        u = UopConfig()
        for lane, s in [
            (1, InpSel.SRC_1), (2, InpSel.CONST_1), (3, InpSel.SRC_0),
            (4, InpSel.CONST_0), (5, InpSel.SRC_0_HI), (6, InpSel.SRC_1_HI),
        ]:
            u.enable_input(s, lane)
        u.require_inp0 = ENABLE
        u.require_inp1 = ENABLE
        u.trigger = (Trigger.SRC_TENSOR_DONE, Trigger.NONE, Trigger.NONE)
        B = u.datapath_config
        # at block 0: PREV_DELAY_n = input lane n+1:
        #   D0=b_lo D1=e2 D2=a_lo D3=s2 D4=a_hi D5=b_hi
        B[0].enable_alu(UAluOp.MIN, AluInp.PREV_DELAY_0, AluInp.PREV_DELAY_1)
        B[0].pass_through_delay(1, 2, 3, 4, 5)          # v_lo in flop
        B[1].enable_alu(UAluOp.MAX, AluInp.PREV_DELAY_2, AluInp.PREV_DELAY_3)
        B[1].enable_delay_from_src(DelayInp.PREV_ALU_OUT, 0)   # D0 <- v_lo
        B[1].pass_through_delay(1, 3, 4, 5)             # u_lo in flop
        B[2].enable_alu(UAluOp.MAX, AluInp.PREV_DELAY_0, AluInp.PREV_ALU_OUT)
        B[2].enable_delay_from_src(DelayInp.PREV_ALU_OUT, 0)   # D0 <- u_lo
        B[2].pass_through_delay(1, 3, 4, 5)             # m_lo in flop
        B[3].enable_alu(UAluOp.SUBTRACT, AluInp.PREV_ALU_OUT, AluInp.PREV_DELAY_0)
        B[3].pass_through_delay(1, 3, 4, 5)             # rw_lo in flop
        B[4].enable_alu(UAluOp.MAX, AluInp.PREV_DELAY_4, AluInp.PREV_DELAY_3)
        B[4].enable_delay_from_src(DelayInp.PREV_ALU_OUT, 0)   # D0 <- rw_lo
        B[4].pass_through_delay(1, 5)                   # u_hi in flop
        B[5].enable_alu(UAluOp.MIN, AluInp.PREV_DELAY_5, AluInp.PREV_DELAY_1)
        B[5].enable_delay_from_src(DelayInp.PREV_ALU_OUT, 2)   # D2 <- u_hi
        B[5].pass_through_delay(0)                      # v_hi in flop
        B[6].enable_alu(UAluOp.MAX, AluInp.PREV_ALU_OUT, AluInp.PREV_DELAY_2)
        B[6].pass_through_delay(0, 2)                   # m_hi in flop
        B[7].enable_alu(UAluOp.SUBTRACT, AluInp.PREV_ALU_OUT, AluInp.PREV_DELAY_2)
        B[7].pass_through_delay(0)                      # rw_hi in flop
        u.enable_output(OutSel.DELAY_0, OutPath.WR0_LO)   # rw_lo
        u.enable_output(OutSel.ALU_OUT, OutPath.WR0_HI)   # rw_hi
        return u

    def _reg(name, spec):
        row = dve_ops._CUSTOM_DVE_ROW_BASE + len(dve_ops.OPS)
        lowered = DveOpSpec(
            name=name, opcode=row, uops=lower(spec, ver=ver),
            rd1_en=_has_src1(spec),
        )
        op = dve_ops.DveOp(
            name, spec, subdim=False, uops_sha={ver: lowered.sha(ver)}
        )
        dve_ops.OPS.append(op)
        dve_ops.CUSTOM_DVE_SPECS[name] = spec
        dve_ops._SUB_OPCODE_FOR_NAME[name] = row
        return op

    # rw = relu(min(b, e2) - max(a, s2)) = max(v, u) - u
    _u = maxx(Src0, C0)
    _v = minn(Src1, C1)
    _rw_body = maxx(_v, _u) - _u

    def _rw_ref(in0, in1, s0, s1, imm2):
        u = np.maximum(in0.astype(np.float32), s0)
        v = np.minimum(in1.astype(np.float32), s1)
        return np.maximum(v, u) - u

    rw4_spec = Spec(body=_rw_body, reference=_rw_ref)
    if os.environ.get("BBK2_RW2X", "1") == "1":
        class _Rw4Op(dve_ops.DveOp):
            def compile(self, v):
                return DveOpSpec(
                    name=self.name,
                    opcode=dve_ops.get_dve_sub_opcode(self.name),
                    uops=lower(self.spec, ver=v),
                    rd1_en=_has_src1(self.spec),
                    uops_2x=[_rw4_2x_uop()],
                    perf_max=1,
                )

        row = dve_ops._CUSTOM_DVE_ROW_BASE + len(dve_ops.OPS)
        rw4 = _Rw4Op("RW4_ANT", rw4_spec, subdim=False, uops_sha={})
        dve_ops.OPS.append(rw4)
        dve_ops.CUSTOM_DVE_SPECS["RW4_ANT"] = rw4_spec
        dve_ops._SUB_OPCODE_FOR_NAME["RW4_ANT"] = row
    else:
        rw4 = _reg("RW4_ANT", rw4_spec)

    # renc ~= 1/enc, enc = max(b, e2) - min(a, s2):
    # seed y0 = bitcast(~enc) * c  lands enc*y0 in [16/17, 18/17] for
    # c = -4/17; one Newton step y1 = y0*(2 - enc*y0) -> |rel err| <= 0.35%
    _mn = minn(Src0, C0)
    _mx = maxx(Src1, C1)
    _enc = _mx - _mn
    _nx = Bin(AluOp.BITWISE_NOT, _enc, _enc)
    _y0 = _nx * C2
    _renc_body = _y0 * ((One + One) - _enc * _y0)

    def _renc_ref(in0, in1, s0, s1, imm2):
        mn = np.minimum(in0.astype(np.float32), s0)
        mx = np.maximum(in1.astype(np.float32), s1)
        enc = (mx - mn).astype(np.float32)
        nx = (~enc.view(np.int32)).view(np.float32)
        y0 = nx * imm2
        return y0 * (2.0 - enc * y0)

    renc8 = _reg("RENC8_ANT", Spec(body=_renc_body, reference=_renc_ref))

    _OPS_REGISTERED.update({"rw4": rw4, "renc8": renc8})
    return _OPS_REGISTERED


def _build_program():
    if not os.environ.get("BBK2_NOPATCH"):
        _patch_act_tables()
    ops = _register_dve_ops()
    rw4, renc8 = ops["rw4"], ops["renc8"]

    nc = bacc.Bacc(
        "TRN2", target_bir_lowering=False, debug=False, enable_asserts=False
    )
    # chunk-major slabs: rows [c*ML, (c+1)*ML) = chunk c
    iou_d = nc.dram_tensor("iou", [NCH * ML, CH], F8, kind="ExternalInput")
    ab_d = nc.dram_tensor("ab", [NCH * ML, 2 * CH], BF16, kind="ExternalInput")
    cm_d = nc.dram_tensor("cm", [NCH * ML, CH], F8, kind="ExternalInput")
    tgt_d = nc.dram_tensor("tgt", [ML, 4], F32, kind="ExternalInput")
    acc_d = nc.dram_tensor("acc", [ML, NCH], F32, kind="ExternalOutput")
    mm_d = nc.dram_tensor("mm", [ML, 2 * BLK], F32, kind="ExternalOutput")
    mmb_d = nc.dram_tensor("mmb", [ML, 2 * BLK], F32, kind="ExternalOutput")

    linearize = bool(int(os.environ.get("BBK2_LINEARIZE", "0")))
    with tile.TileContext(nc, linearize=linearize) as tc:
        with (
            tc.tile_pool(name="const", bufs=1) as cp,
            tc.tile_pool(name="inp", bufs=int(os.environ.get("BBK2_IBUFS", "4"))) as ip,
            tc.tile_pool(name="work", bufs=int(os.environ.get("BBK2_WBUFS", "4"))) as wp,
            tc.psum_pool(name="ps", bufs=1) as pp,
        ):
            tgt = cp.tile([ML, 4], F32)
            s2 = tgt[:, 0:1]
            e2 = tgt[:, 1:2]
            nc2 = tgt[:, 2:3]          # -c2 (ACT Identity bias)
            sgb = tgt[:, 3:4]          # -2048 (sigmoid step bias)

            acc = cp.tile([ML, NCH], F32)
            pb1 = pp.tile([ML, BLK], F32)
            pb2 = pp.tile([ML, BLK], F32)
            pb1b = pp.tile([ML, BLK], F32)
            pb2b = pp.tile([ML, BLK], F32)

            # dummy activation: hoists ACT_TABLE_LOAD to t~0, concurrent
            # with the first input DMAs
            warm = cp.tile([ML, 1], F32)
            nc.vector.memset(warm[:], 0.0)
            nc.scalar.activation(warm[:], warm[:], AF.Sigmoid, bias=0.0, scale=1.0)

            nc.sync.dma_start(tgt[:], tgt_d.ap())

            nblk = (CH + BLK - 1) // BLK
            for c in range(NCH):
                abt = ip.tile([ML, 2 * CH], BF16, tag="abt")
                nc.sync.dma_start(abt[:], ab_d.ap()[c * ML : (c + 1) * ML, :])
                iot = ip.tile([ML, CH], F8, tag="iot")
                nc.sync.dma_start(iot[:], iou_d.ap()[c * ML : (c + 1) * ML, :])
                ct = ip.tile([ML, CH], F8, tag="ct")
                nc.sync.dma_start(ct[:], cm_d.ap()[c * ML : (c + 1) * ML, :])
                a = abt[:, 0:CH]           # s1
                b = abt[:, CH : 2 * CH]    # e1
                c1 = ct[:]

                # mask = hard step(iou > 0.5), fused row accumulate -> A
                mask = wp.tile([ML, CH], BF16, tag="mask")
                nc.scalar.activation(
                    mask[:], iot[:], AF.Sigmoid,
                    bias=sgb, scale=4096.0,
                    accum_out=acc[:, c : c + 1],
                )

                rw = wp.tile([ML, CH], BF16, tag="rw")
                _bi = nc.vector._custom_dve(
                    rw4, out=rw[:], in0=a, in1=b, s0=s2, s1=e2
                )
                for _attr in ("ins", "instruction", "inst"):
                    _obj = getattr(_bi, _attr, None)
                    if _obj is not None and hasattr(_obj, "perf_max"):
                        _obj.perf_max = 1
                        break
                else:
                    if hasattr(_bi, "perf_max"):
                        _bi.perf_max = 1
                renc = wp.tile([ML, CH], BF16, tag="renc")
                nc.vector._custom_dve(
                    renc8, out=renc[:], in0=a, in1=b, s0=s2, s1=e2,
                    imm2=-4.0 / 17.0,
                )

                rm = wp.tile([ML, CH], BF16, tag="rm")
                nc.vector.tensor_tensor(rm[:], mask[:], renc[:], OP.mult)

                cd = wp.tile([ML, CH], BF16, tag="cd")
                nc.scalar.activation(
                    cd[:], c1, AF.Identity, bias=nc2, scale=1.0
                )

                qm = wp.tile([ML, CH], BF16, tag="qm")
                nc.vector.tensor_tensor(qm[:], cd[:], rm[:], OP.mult)

                # chunks 0..NCH-2 accumulate into pb1/pb2; the last chunk
                # into pb1b/pb2b so the big accumulators drain (copy + DMA)
                # while the last chunk computes
                t1 = pb1 if c < NCH - 1 else pb1b
                t2 = pb2 if c < NCH - 1 else pb2b
                for bi in range(nblk):
                    lo = bi * BLK
                    hi = min(CH, lo + BLK)
                    wb = hi - lo
                    first = c in (0, NCH - 1) and bi == 0
                    last = c in (NCH - 2, NCH - 1) and bi == nblk - 1
                    nc.tensor.matmul(
                        t1[0:wb, 0:wb], rw[:, lo:hi], rm[:, lo:hi],
                        start=first, stop=last,
                    )
                    nc.tensor.matmul(
                        t2[0:wb, 0:wb], qm[:, lo:hi], qm[:, lo:hi],
                        start=first, stop=last,
                    )
                if c == NCH - 2:
                    mma = cp.tile([ML, 2 * BLK], F32)
                    nc.scalar.copy(mma[:, 0:BLK], pb1[:])
                    nc.scalar.copy(mma[:, BLK : 2 * BLK], pb2[:])
                    nc.sync.dma_start(mm_d.ap(), mma[:])

            mmb = cp.tile([ML, 2 * BLK], F32)
            nc.scalar.copy(mmb[:, 0:BLK], pb1b[:])
            nc.scalar.copy(mmb[:, BLK : 2 * BLK], pb2b[:])
            nc.sync.dma_start(mmb_d.ap(), mmb[:])
            nc.sync.dma_start(acc_d.ap(), acc[:])

    nc.compile()
    return nc


_NC_CACHE = None


def _get_program():
    global _NC_CACHE
    if _NC_CACHE is None:
        _NC_CACHE = _build_program()
    return _NC_CACHE


def _reference_numpy(out_moments, tgt_moments, num_targets, iou2ds, mask2d):
    """Exact numpy replica of the jax reference (fallback path)."""
    M_, N_, _ = iou2ds.shape
    S_, P_, _ = out_moments.shape
    scatter = np.repeat(np.arange(S_), num_targets)
    om = out_moments[scatter].astype(np.float32)      # [M, P, 2]
    tg = tgt_moments[:, None, :].astype(np.float32)
    s1, e1 = om[..., 0], om[..., 1]
    s2, e2 = tg[..., 0], tg[..., 1]
    inter = np.clip(np.minimum(e1, e2) - np.maximum(s1, s2), 0.0, None)
    union = (e1 - s1) + (e2 - s2) - inter
    iou = inter / union
    enclose = np.maximum(e1, e2) - np.minimum(s1, s2)
    cdist = (s1 + e1) * 0.5 - (s2 + e2) * 0.5
    bbox_diou = iou - (cdist * cdist) / (enclose * enclose)
    flat_idx = np.nonzero(mask2d.reshape(-1))[0]
    iou1 = iou2ds.reshape(M_, -1)[:, flat_idx]
    kth = np.argpartition(-iou1, TOPK - 1, axis=1)[:, :TOPK]
    target_mask = np.zeros((M_, P_), np.float32)
    target_mask[np.arange(M_)[:, None], kth] = 1.0
    target_mask = np.where(iou1 > 0.5, 1.0, target_mask)
    loss = 1.0 - bbox_diou
    return np.float32((loss * target_mask).sum() / target_mask.sum())


def kernel(out_moments, tgt_moments, num_targets, iou2ds, mask2d):
    out_moments = np.asarray(out_moments, np.float32)
    tgt_moments = np.asarray(tgt_moments, np.float32)
    num_targets = np.asarray(num_targets, np.int32)
    iou2ds = np.asarray(iou2ds, np.float32)
    mask2d_np = np.asarray(mask2d)

    uniform = bool(np.all(num_targets == T))
    triu_ok = bool(
        np.array_equal(mask2d_np, np.triu(np.ones((N, N), dtype=bool)))
    )
    if not (uniform and triu_ok and iou2ds.shape == (M, N, N)):
        return _reference_numpy(
            out_moments, tgt_moments, num_targets, iou2ds, mask2d_np
        )

    nc = _get_program()
    f8 = ml_dtypes.float8_e4m3

    # host layout prep: triu-compact iou2ds to p-order, fp8
    flat_idx = np.nonzero(mask2d_np.reshape(-1))[0]
    iou1 = iou2ds.reshape(M, -1)[:, flat_idx].astype(f8)     # [M, P]
    s1 = out_moments[..., 0]                                  # [S, P] f32
    e1 = out_moments[..., 1]
    bf16 = ml_dtypes.bfloat16
    c1 = (s1 + e1).astype(f8)
    s1 = s1.astype(bf16)
    e1 = e1.astype(bf16)

    in_maps = []
    for k in range(NCORES):
        sl_m = slice(k * ML, (k + 1) * ML)
        sl_s = slice(k * W, (k + 1) * W)
        # replicate each sample's moments across its 16 target partitions
        s1k = np.repeat(s1[sl_s], T, axis=0)                  # [128, P]
        e1k = np.repeat(e1[sl_s], T, axis=0)
        c1k = np.repeat(c1[sl_s], T, axis=0)
        iouk = iou1[sl_m]
        pio = np.empty((NCH, ML, CH), f8)
        pab = np.empty((NCH, ML, 2 * CH), bf16)
        pcm = np.empty((NCH, ML, CH), f8)
        for c in range(NCH):
            sl_p = slice(c * CH, (c + 1) * CH)
            pio[c] = iouk[:, sl_p]
            pab[c, :, 0:CH] = s1k[:, sl_p]
            pab[c, :, CH : 2 * CH] = e1k[:, sl_p]
            pcm[c] = c1k[:, sl_p]
        tgtk = tgt_moments[sl_m]                              # [128, 2] f32
        tgt4 = np.zeros((ML, 4), np.float32)
        tgt4[:, 0] = tgtk[:, 0]
        tgt4[:, 1] = tgtk[:, 1]
        tgt4[:, 2] = -(tgtk[:, 0] + tgtk[:, 1])               # -c2
        tgt4[:, 3] = -2048.0                                  # sigmoid bias
        in_maps.append(
            {
                "iou": np.ascontiguousarray(pio.reshape(NCH * ML, CH)),
                "ab": np.ascontiguousarray(pab.reshape(NCH * ML, 2 * CH)),
                "cm": np.ascontiguousarray(pcm.reshape(NCH * ML, CH)),
                "tgt": tgt4,
            }
        )

    trace = bool(int(os.environ.get("BBK_TRACE", "0")))
    res = bass_utils.run_bass_kernel_spmd(
        nc, in_maps, core_ids=list(range(NCORES)), trace=trace
    )
    if trace:
        kernel.last_exec_time_ns = res.exec_time_ns

    acc = np.stack([res.results[k]["acc"] for k in range(NCORES)])  # [8,128,NCH]
    mm = np.stack(
        [
            res.results[k]["mm"].astype(np.float64)
            + res.results[k]["mmb"].astype(np.float64)
            for k in range(NCORES)
        ]
    )
    acc64 = acc.astype(np.float64)
    a_rows = acc64.sum(axis=2)                     # per-core per-row counts
    A = a_rows.sum()
    mm64 = mm.astype(np.float64)
    B1 = np.trace(mm64[:, :, 0:BLK], axis1=1, axis2=2).sum()
    B2 = np.trace(mm64[:, :, BLK : 2 * BLK], axis1=1, axis2=2).sum() / 4.0

    if a_rows.min() < 4 * TOPK:
        # threshold may not subsume top-3: replicate reference on host
        return _reference_numpy(
            out_moments, tgt_moments, num_targets, iou2ds, mask2d_np
        )

    return np.float32((A - B1 + B2) / A)


# revision 4
# speedup vs baseline: 1.1107x; 1.0355x over previous
"""Bass/Trainium2 kernel v3 for nn_BboxIoULoss (topk_masking).

loss = sum((1 - diou) * mask) / sum(mask),  mask = (iou1ds > 0.5) OR top-3
(top-3 subsumed by the threshold on these inputs; checked, numpy fallback
otherwise).

Strategy (8 cores data-parallel over M; per core 128 target partitions):
  - all four input slabs (iou, s1, e1, c1=s1+e1) ship as fp8 e4m3
    (4.2 MB/core): B1/B2 enter the loss scaled by ~0.019 so %-level error
    there moves the answer ~1e-3 << 2e-2 tolerance.
  - ACT: mask = Sigmoid(4096*iou - 2048) (hard 0/1 step at the fp8 grid
    point 0.5; values == 0.5 get weight 1/2, statistically neutral) with
    fused row-accumulate -> A; cd = c1 - c2 via Identity + per-partition
    bias.  A tiny warm-up activation hoists ACT_TABLE_LOAD to t~0.
  - DVE: two runtime-registered custom fused ops
      RW4_ANT:   rw  = relu(min(e1,e2) - max(s1,s2))        [4 stages]
      RENC8_ANT: renc ~= 1/(max(e1,e2) - min(s1,s2))        [8 stages]
        (enclose + bitwise-NOT reciprocal seed + 1 Newton step, ~0.4%)
    plus stock 2x-mode rm = mask*renc and qm = cd*rm.
  - TensorE: block-diagonal gram accumulation
      PB1 += rw_blk^T @ rm_blk   (trace = B1 = sum mask*iou)
      PB2 += qm_blk^T @ qm_blk   (trace = 4*B2 = 4*sum mask*pen)
  - answer = (A - B1 + B2) / A   (host sums traces in float64)
"""

import os
import ml_dtypes
import numpy as np

import concourse.bass as bass
import concourse.tile as tile
import concourse.mybir as mybir
from concourse import bacc, bass_utils

F32 = mybir.dt.float32
BF16 = mybir.dt.bfloat16
F8 = mybir.dt.float8e4
AF = mybir.ActivationFunctionType
OP = mybir.AluOpType

S = 64
T = 16
N = 128
M = S * T                  # 1024
P = N * (N + 1) // 2       # 8256
TOPK = 3
NCORES = 8
ML = M // NCORES           # 128 targets / core (= partitions)
W = S // NCORES            # 8 samples / core
NCH = int(os.environ.get("BBK2_NCH", "6"))
CH = P // NCH              # free-dim chunk
BLK = 128                  # matmul block width


def _patch_act_tables():
    """Pin one activation table-set (sigmoid/identity/copy) so the
    scheduler emits a single ACT_TABLE_LOAD."""
    import functools
    import concourse.hw_specs as _hw

    orig = _hw.get_activation_tables.__wrapped__

    def only_sigmoid(arch):
        tabs = orig(arch)
        name = "sigmoid_and_others"
        if name not in tabs:
            return tabs
        return {k: (v if k == name else set()) for k, v in tabs.items()}

    _hw.get_activation_tables = functools.cache(only_sigmoid)
    bacc.get_activation_tables = _hw.get_activation_tables


_OPS_REGISTERED = {}


def _register_dve_ops():
    """Register the two fused DVE ops at runtime (dve_ops.py is read-only)."""
    if _OPS_REGISTERED:
        return _OPS_REGISTERED
    import concourse.dve_ops as dve_ops
    from concourse.dve_spec import (
        Spec, Src0, Src1, C0, C1, C2, One, maxx, minn, lower, _has_src1,
        Bin, AluOp,
    )
    from concourse.dve_uop import (
        DveOpSpec, UopConfig, UopDpConfig, Trigger, InpSel, OutSel, OutPath,
        AluInp, DelayInp, ENABLE, AluOp as UAluOp,
    )
    from concourse.dve_table_gen import dve_ver_for

    ver = dve_ver_for("TRN2")

    def _rw4_2x_uop():
        """Hand-packed 2x_1P program for RW4: lo chain on blocks 0-3
        (result rides delay lane 0 to the end), hi chain on blocks 4-7
        (result in block 7's ALU flop).  Mirrors the stock tensor_tensor
        2x slot conventions (SRC_*_HI input lanes, write0_en_lo+hi)."""
        u = UopConfig()
        for lane, s in [
            (1, InpSel.SRC_1), (2, InpSel.CONST_1), (3, InpSel.SRC_0),
            (4, InpSel.CONST_0), (5, InpSel.SRC_0_HI), (6, InpSel.SRC_1_HI),
        ]:
            u.enable_input(s, lane)
        u.require_inp0 = ENABLE
        u.require_inp1 = ENABLE
        u.trigger = (Trigger.SRC_TENSOR_DONE, Trigger.NONE, Trigger.NONE)
        B = u.datapath_config
        # at block 0: PREV_DELAY_n = input lane n+1:
        #   D0=b_lo D1=e2 D2=a_lo D3=s2 D4=a_hi D5=b_hi
        B[0].enable_alu(UAluOp.MIN, AluInp.PREV_DELAY_0, AluInp.PREV_DELAY_1)
        B[0].pass_through_delay(1, 2, 3, 4, 5)          # v_lo in flop
        B[1].enable_alu(UAluOp.MAX, AluInp.PREV_DELAY_2, AluInp.PREV_DELAY_3)
        B[1].enable_delay_from_src(DelayInp.PREV_ALU_OUT, 0)   # D0 <- v_lo
        B[1].pass_through_delay(1, 3, 4, 5)             # u_lo in flop
        B[2].enable_alu(UAluOp.MAX, AluInp.PREV_DELAY_0, AluInp.PREV_ALU_OUT)
        B[2].enable_delay_from_src(DelayInp.PREV_ALU_OUT, 0)   # D0 <- u_lo
        B[2].pass_through_delay(1, 3, 4, 5)             # m_lo in flop
        B[3].enable_alu(UAluOp.SUBTRACT, AluInp.PREV_ALU_OUT, AluInp.PREV_DELAY_0)
        B[3].pass_through_delay(1, 3, 4, 5)             # rw_lo in flop
        B[4].enable_alu(UAluOp.MAX, AluInp.PREV_DELAY_4, AluInp.PREV_DELAY_3)
        B[4].enable_delay_from_src(DelayInp.PREV_ALU_OUT, 0)   # D0 <- rw_lo
        B[4].pass_through_delay(1, 5)                   # u_hi in flop
        B[5].enable_alu(UAluOp.MIN, AluInp.PREV_DELAY_5, AluInp.PREV_DELAY_1)
        B[5].enable_delay_from_src(DelayInp.PREV_ALU_OUT, 2)   # D2 <- u_hi
        B[5].pass_through_delay(0)                      # v_hi in flop
        B[6].enable_alu(UAluOp.MAX, AluInp.PREV_ALU_OUT, AluInp.PREV_DELAY_2)
        B[6].pass_through_delay(0, 2)                   # m_hi in flop
        B[7].enable_alu(UAluOp.SUBTRACT, AluInp.PREV_ALU_OUT, AluInp.PREV_DELAY_2)
        B[7].pass_through_delay(0)                      # rw_hi in flop
        u.enable_output(OutSel.DELAY_0, OutPath.WR0_LO)   # rw_lo
        u.enable_output(OutSel.ALU_OUT, OutPath.WR0_HI)   # rw_hi
        return u

    def _reg(name, spec):
        row = dve_ops._CUSTOM_DVE_ROW_BASE + len(dve_ops.OPS)
        lowered = DveOpSpec(
            name=name, opcode=row, uops=lower(spec, ver=ver),
            rd1_en=_has_src1(spec),
        )
        op = dve_ops.DveOp(
            name, spec, subdim=False, uops_sha={ver: lowered.sha(ver)}
        )
        dve_ops.OPS.append(op)
        dve_ops.CUSTOM_DVE_SPECS[name] = spec
        dve_ops._SUB_OPCODE_FOR_NAME[name] = row
        return op

    # rw = relu(min(b, e2) - max(a, s2)) = max(v, u) - u
    _u = maxx(Src0, C0)
    _v = minn(Src1, C1)
    _rw_body = maxx(_v, _u) - _u

    def _rw_ref(in0, in1, s0, s1, imm2):
        u = np.maximum(in0.astype(np.float32), s0)
        v = np.minimum(in1.astype(np.float32), s1)
        return np.maximum(v, u) - u

    def _reg2x(name, spec, uop2x_fn):
        class _PerfOp(dve_ops.DveOp):
            def compile(self, v):
                return DveOpSpec(
                    name=self.name,
                    opcode=dve_ops.get_dve_sub_opcode(self.name),
                    uops=lower(self.spec, ver=v),
                    rd1_en=_has_src1(self.spec),
                    uops_2x=[uop2x_fn()],
                    perf_max=1,
                )

        row = dve_ops._CUSTOM_DVE_ROW_BASE + len(dve_ops.OPS)
        op = _PerfOp(name, spec, subdim=False, uops_sha={})
        dve_ops.OPS.append(op)
        dve_ops.CUSTOM_DVE_SPECS[name] = spec
        dve_ops._SUB_OPCODE_FOR_NAME[name] = row
        return op

    rw4_spec = Spec(body=_rw_body, reference=_rw_ref)
    if os.environ.get("BBK2_RW2X", "1") == "1":
        rw4 = _reg2x("RW4_ANT", rw4_spec, _rw4_2x_uop)
    else:
        rw4 = _reg("RW4_ANT", rw4_spec)

    # nx = bitcast(~enc) ~= -4.25/enc, enc = max(b, e2) - min(a, s2).
    # The reciprocal seed scale (-4/17, so enc*(nx*c) lands within +-5.9%
    # of 1) is folded into the RM3 pass below; end-to-end loss error from
    # the seed-only reciprocal is ~1e-3 (measured), << 2e-2.
    _mn = minn(Src0, C0)
    _mx = maxx(Src1, C1)
    _enc = _mx - _mn
    _nx_body = Bin(AluOp.BITWISE_NOT, _enc, _enc)

    def _renc_ref(in0, in1, s0, s1, imm2):
        mn = np.minimum(in0.astype(np.float32), s0)
        mx = np.maximum(in1.astype(np.float32), s1)
        enc = (mx - mn).astype(np.float32)
        return (~enc.view(np.int32)).view(np.float32)

    def _renc4_2x_uop():
        u = UopConfig()
        for lane, s in [
            (1, InpSel.SRC_1), (2, InpSel.CONST_1), (3, InpSel.SRC_0),
            (4, InpSel.CONST_0), (5, InpSel.SRC_0_HI), (6, InpSel.SRC_1_HI),
        ]:
            u.enable_input(s, lane)
        u.require_inp0 = ENABLE
        u.require_inp1 = ENABLE
        u.trigger = (Trigger.SRC_TENSOR_DONE, Trigger.NONE, Trigger.NONE)
        B = u.datapath_config
        # block 0 PREV_DELAY_n = lane n+1: D0=b_lo D1=e2 D2=a_lo D3=s2
        #                                  D4=a_hi D5=b_hi
        B[0].enable_alu(UAluOp.MAX, AluInp.PREV_DELAY_0, AluInp.PREV_DELAY_1)
        B[0].pass_through_delay(1, 2, 3, 4, 5)          # mx_lo in flop
        B[1].enable_alu(UAluOp.MIN, AluInp.PREV_DELAY_2, AluInp.PREV_DELAY_3)
        B[1].enable_delay_from_src(DelayInp.PREV_ALU_OUT, 0)   # D0 <- mx_lo
        B[1].pass_through_delay(1, 3, 4, 5)             # mn_lo in flop
        B[2].enable_alu(UAluOp.SUBTRACT, AluInp.PREV_DELAY_0, AluInp.PREV_ALU_OUT)
        B[2].pass_through_delay(1, 3, 4, 5)             # enc_lo in flop
        B[3].enable_alu(UAluOp.BITWISE_NOT, AluInp.PREV_ALU_OUT, AluInp.PREV_ALU_OUT)
        B[3].pass_through_delay(1, 3, 4, 5)             # nx_lo in flop
        B[4].enable_alu(UAluOp.MAX, AluInp.PREV_DELAY_5, AluInp.PREV_DELAY_1)
        B[4].enable_delay_from_src(DelayInp.PREV_ALU_OUT, 0)   # D0 <- nx_lo
        B[4].pass_through_delay(3, 4)                   # mx_hi in flop
        B[5].enable_alu(UAluOp.MIN, AluInp.PREV_DELAY_4, AluInp.PREV_DELAY_3)
        B[5].enable_delay_from_src(DelayInp.PREV_ALU_OUT, 2)   # D2 <- mx_hi
        B[5].pass_through_delay(0)                      # mn_hi in flop
        B[6].enable_alu(UAluOp.SUBTRACT, AluInp.PREV_DELAY_2, AluInp.PREV_ALU_OUT)
        B[6].pass_through_delay(0)                      # enc_hi in flop
        B[7].enable_alu(UAluOp.BITWISE_NOT, AluInp.PREV_ALU_OUT, AluInp.PREV_ALU_OUT)
        B[7].pass_through_delay(0)                      # nx_hi in flop
        u.enable_output(OutSel.DELAY_0, OutPath.WR0_LO)   # nx_lo
        u.enable_output(OutSel.ALU_OUT, OutPath.WR0_HI)   # nx_hi
        return u

    renc4 = _reg2x(
        "RENC4_ANT", Spec(body=_nx_body, reference=_renc_ref), _renc4_2x_uop
    )

    # rm = mask * (nx * c0): the reciprocal-seed scale rides this pass
    _rm_body = (Src1 * C0) * Src0

    def _rm_ref(in0, in1, s0, s1, imm2):
        return (in1.astype(np.float32) * s0) * in0.astype(np.float32)

    def _rm3_2x_uop():
        u = UopConfig()
        for lane, s in [
            (1, InpSel.SRC_1), (2, InpSel.CONST_0), (3, InpSel.SRC_0),
            (4, InpSel.SRC_1_HI), (5, InpSel.SRC_0_HI),
        ]:
            u.enable_input(s, lane)
        u.require_inp0 = ENABLE
        u.require_inp1 = ENABLE
        u.trigger = (Trigger.SRC_TENSOR_DONE, Trigger.NONE, Trigger.NONE)
        B = u.datapath_config
        # block 0 PREV_DELAY_n = lane n+1: D0=nx_lo D1=c D2=mask_lo
        #                                  D3=nx_hi D4=mask_hi
        B[0].enable_alu(UAluOp.MULTIPLY, AluInp.PREV_DELAY_0, AluInp.PREV_DELAY_1)
        B[0].pass_through_delay(1, 2, 3, 4)             # t_lo in flop
        B[1].enable_alu(UAluOp.MULTIPLY, AluInp.PREV_ALU_OUT, AluInp.PREV_DELAY_2)
        B[1].pass_through_delay(1, 3, 4)                # rm_lo in flop
        B[2].enable_alu(UAluOp.MULTIPLY, AluInp.PREV_DELAY_3, AluInp.PREV_DELAY_1)
        B[2].enable_delay_from_src(DelayInp.PREV_ALU_OUT, 0)   # D0 <- rm_lo
        B[2].pass_through_delay(4)                      # t_hi in flop
        B[3].enable_alu(UAluOp.MULTIPLY, AluInp.PREV_ALU_OUT, AluInp.PREV_DELAY_4)
        B[3].pass_through_delay(0)                      # rm_hi in flop
        for k in (4, 5, 6, 7):
            B[k].pass_through_alu()                     # carry rm_hi
            B[k].pass_through_delay(0)                  # carry rm_lo
        u.enable_output(OutSel.DELAY_0, OutPath.WR0_LO)   # rm_lo
        u.enable_output(OutSel.ALU_OUT, OutPath.WR0_HI)   # rm_hi
        return u

    rm3 = _reg2x(
        "RM3_ANT", Spec(body=_rm_body, reference=_rm_ref), _rm3_2x_uop
    )

    _OPS_REGISTERED.update({"rw4": rw4, "renc4": renc4, "rm3": rm3})
    return _OPS_REGISTERED


def _build_program():
    if not os.environ.get("BBK2_NOPATCH"):
        _patch_act_tables()
    ops = _register_dve_ops()
    rw4, renc4, rm3 = ops["rw4"], ops["renc4"], ops["rm3"]

    nc = bacc.Bacc(
        "TRN2", target_bir_lowering=False, debug=False, enable_asserts=False
    )
    # chunk-major slabs: rows [c*ML, (c+1)*ML) = chunk c
    iou_d = nc.dram_tensor("iou", [NCH * ML, CH], F8, kind="ExternalInput")
    ab_d = nc.dram_tensor("ab", [NCH * ML, 2 * CH], BF16, kind="ExternalInput")
    cm_d = nc.dram_tensor("cm", [NCH * ML, CH], F8, kind="ExternalInput")
    tgt_d = nc.dram_tensor("tgt", [ML, 4], F32, kind="ExternalInput")
    acc_d = nc.dram_tensor("acc", [ML, NCH], F32, kind="ExternalOutput")
    mm_d = nc.dram_tensor("mm", [ML, 2 * BLK], F32, kind="ExternalOutput")
    mmb_d = nc.dram_tensor("mmb", [ML, 2 * BLK], F32, kind="ExternalOutput")

    linearize = bool(int(os.environ.get("BBK2_LINEARIZE", "0")))
    with tile.TileContext(nc, linearize=linearize) as tc:
        with (
            tc.tile_pool(name="const", bufs=1) as cp,
            tc.tile_pool(name="inp", bufs=int(os.environ.get("BBK2_IBUFS", "4"))) as ip,
            tc.tile_pool(name="work", bufs=int(os.environ.get("BBK2_WBUFS", "4"))) as wp,
            tc.psum_pool(name="ps", bufs=1) as pp,
        ):
            tgt = cp.tile([ML, 4], F32)
            s2 = tgt[:, 0:1]
            e2 = tgt[:, 1:2]
            nc2 = tgt[:, 2:3]          # -c2 (ACT Identity bias)
            sgb = tgt[:, 3:4]          # -2048 (sigmoid step bias)

            acc = cp.tile([ML, NCH], F32)
            pb1 = pp.tile([ML, BLK], F32)
            pb2 = pp.tile([ML, BLK], F32)
            pb1b = pp.tile([ML, BLK], F32)
            pb2b = pp.tile([ML, BLK], F32)

            # dummy activation: hoists ACT_TABLE_LOAD to t~0, concurrent
            # with the first input DMAs
            warm = cp.tile([ML, 1], F32)
            nc.vector.memset(warm[:], 0.0)
            nc.scalar.activation(warm[:], warm[:], AF.Sigmoid, bias=0.0, scale=1.0)

            nc.sync.dma_start(tgt[:], tgt_d.ap())

            nblk = (CH + BLK - 1) // BLK
            for c in range(NCH):
                abt = ip.tile([ML, 2 * CH], BF16, tag="abt")
                nc.sync.dma_start(abt[:], ab_d.ap()[c * ML : (c + 1) * ML, :])
                iot = ip.tile([ML, CH], F8, tag="iot")
                nc.sync.dma_start(iot[:], iou_d.ap()[c * ML : (c + 1) * ML, :])
                ct = ip.tile([ML, CH], F8, tag="ct")
                nc.sync.dma_start(ct[:], cm_d.ap()[c * ML : (c + 1) * ML, :])
                a = abt[:, 0:CH]           # s1
                b = abt[:, CH : 2 * CH]    # e1
                c1 = ct[:]

                # mask = hard step(iou > 0.5), fused row accumulate -> A
                mask = wp.tile([ML, CH], BF16, tag="mask")
                nc.scalar.activation(
                    mask[:], iot[:], AF.Sigmoid,
                    bias=sgb, scale=4096.0,
                    accum_out=acc[:, c : c + 1],
                )

                rw = wp.tile([ML, CH], BF16, tag="rw")
                _bi = nc.vector._custom_dve(
                    rw4, out=rw[:], in0=a, in1=b, s0=s2, s1=e2
                )
                for _attr in ("ins", "instruction", "inst"):
                    _obj = getattr(_bi, _attr, None)
                    if _obj is not None and hasattr(_obj, "perf_max"):
                        _obj.perf_max = 1
                        break
                else:
                    if hasattr(_bi, "perf_max"):
                        _bi.perf_max = 1
                renc = wp.tile([ML, CH], BF16, tag="renc")
                _b2 = nc.vector._custom_dve(
                    renc4, out=renc[:], in0=a, in1=b, s0=s2, s1=e2
                )
                rm = wp.tile([ML, CH], BF16, tag="rm")
                _b3 = nc.vector._custom_dve(
                    rm3, out=rm[:], in0=mask[:], in1=renc[:], s0=-4.0 / 17.0
                )
                for _o in (_b2, _b3):
                    for _attr in ("ins", "instruction", "inst"):
                        _obj = getattr(_o, _attr, None)
                        if _obj is not None and hasattr(_obj, "perf_max"):
                            _obj.perf_max = 1
                            break
                    else:
                        if hasattr(_o, "perf_max"):
                            _o.perf_max = 1

                cd = wp.tile([ML, CH], BF16, tag="cd")
                nc.scalar.activation(
                    cd[:], c1, AF.Identity, bias=nc2, scale=1.0
                )

                qm = wp.tile([ML, CH], BF16, tag="qm")
                nc.vector.tensor_tensor(qm[:], cd[:], rm[:], OP.mult)

                # chunks 0..NCH-2 accumulate into pb1/pb2; the last chunk
                # into pb1b/pb2b so the big accumulators drain (copy + DMA)
                # while the last chunk computes
                t1 = pb1 if c < NCH - 1 else pb1b
                t2 = pb2 if c < NCH - 1 else pb2b
                for bi in range(nblk):
                    lo = bi * BLK
                    hi = min(CH, lo + BLK)
                    wb = hi - lo
                    first = c in (0, NCH - 1) and bi == 0
                    last = c in (NCH - 2, NCH - 1) and bi == nblk - 1
                    nc.tensor.matmul(
                        t1[0:wb, 0:wb], rw[:, lo:hi], rm[:, lo:hi],
                        start=first, stop=last,
                    )
                    nc.tensor.matmul(
                        t2[0:wb, 0:wb], qm[:, lo:hi], qm[:, lo:hi],
                        start=first, stop=last,
                    )
                if c == NCH - 2:
                    mma = cp.tile([ML, 2 * BLK], F32)
                    nc.scalar.copy(mma[:, 0:BLK], pb1[:])
                    nc.scalar.copy(mma[:, BLK : 2 * BLK], pb2[:])
                    nc.sync.dma_start(mm_d.ap(), mma[:])

            mmb = cp.tile([ML, 2 * BLK], F32)
            nc.scalar.copy(mmb[:, 0:BLK], pb1b[:])
            nc.scalar.copy(mmb[:, BLK : 2 * BLK], pb2b[:])
            nc.sync.dma_start(mmb_d.ap(), mmb[:])
            nc.sync.dma_start(acc_d.ap(), acc[:])

    nc.compile()
    return nc


_NC_CACHE = None


def _get_program():
    global _NC_CACHE
    if _NC_CACHE is None:
        _NC_CACHE = _build_program()
    return _NC_CACHE


def _reference_numpy(out_moments, tgt_moments, num_targets, iou2ds, mask2d):
    """Exact numpy replica of the jax reference (fallback path)."""
    M_, N_, _ = iou2ds.shape
    S_, P_, _ = out_moments.shape
    scatter = np.repeat(np.arange(S_), num_targets)
    om = out_moments[scatter].astype(np.float32)      # [M, P, 2]
    tg = tgt_moments[:, None, :].astype(np.float32)
    s1, e1 = om[..., 0], om[..., 1]
    s2, e2 = tg[..., 0], tg[..., 1]
    inter = np.clip(np.minimum(e1, e2) - np.maximum(s1, s2), 0.0, None)
    union = (e1 - s1) + (e2 - s2) - inter
    iou = inter / union
    enclose = np.maximum(e1, e2) - np.minimum(s1, s2)
    cdist = (s1 + e1) * 0.5 - (s2 + e2) * 0.5
    bbox_diou = iou - (cdist * cdist) / (enclose * enclose)
    flat_idx = np.nonzero(mask2d.reshape(-1))[0]
    iou1 = iou2ds.reshape(M_, -1)[:, flat_idx]
    kth = np.argpartition(-iou1, TOPK - 1, axis=1)[:, :TOPK]
    target_mask = np.zeros((M_, P_), np.float32)
    target_mask[np.arange(M_)[:, None], kth] = 1.0
    target_mask = np.where(iou1 > 0.5, 1.0, target_mask)
    loss = 1.0 - bbox_diou
    return np.float32((loss * target_mask).sum() / target_mask.sum())


def kernel(out_moments, tgt_moments, num_targets, iou2ds, mask2d):
    out_moments = np.asarray(out_moments, np.float32)
    tgt_moments = np.asarray(tgt_moments, np.float32)
    num_targets = np.asarray(num_targets, np.int32)
    iou2ds = np.asarray(iou2ds, np.float32)
    mask2d_np = np.asarray(mask2d)

    uniform = bool(np.all(num_targets == T))
    triu_ok = bool(
        np.array_equal(mask2d_np, np.triu(np.ones((N, N), dtype=bool)))
    )
    if not (uniform and triu_ok and iou2ds.shape == (M, N, N)):
        return _reference_numpy(
            out_moments, tgt_moments, num_targets, iou2ds, mask2d_np
        )

    nc = _get_program()
    f8 = ml_dtypes.float8_e4m3

    # host layout prep: triu-compact iou2ds to p-order, fp8
    flat_idx = np.nonzero(mask2d_np.reshape(-1))[0]
    iou1 = iou2ds.reshape(M, -1)[:, flat_idx].astype(f8)     # [M, P]
    s1 = out_moments[..., 0]                                  # [S, P] f32
    e1 = out_moments[..., 1]
    bf16 = ml_dtypes.bfloat16
    c1 = (s1 + e1).astype(f8)
    s1 = s1.astype(bf16)
    e1 = e1.astype(bf16)

    in_maps = []
    for k in range(NCORES):
        sl_m = slice(k * ML, (k + 1) * ML)
        sl_s = slice(k * W, (k + 1) * W)
        # replicate each sample's moments across its 16 target partitions
        s1k = np.repeat(s1[sl_s], T, axis=0)                  # [128, P]
        e1k = np.repeat(e1[sl_s], T, axis=0)
        c1k = np.repeat(c1[sl_s], T, axis=0)
        iouk = iou1[sl_m]
        pio = np.empty((NCH, ML, CH), f8)
        pab = np.empty((NCH, ML, 2 * CH), bf16)
        pcm = np.empty((NCH, ML, CH), f8)
        for c in range(NCH):
            sl_p = slice(c * CH, (c + 1) * CH)
            pio[c] = iouk[:, sl_p]
            pab[c, :, 0:CH] = s1k[:, sl_p]
            pab[c, :, CH : 2 * CH] = e1k[:, sl_p]
            pcm[c] = c1k[:, sl_p]
        tgtk = tgt_moments[sl_m]                              # [128, 2] f32
        tgt4 = np.zeros((ML, 4), np.float32)
        tgt4[:, 0] = tgtk[:, 0]
        tgt4[:, 1] = tgtk[:, 1]
        tgt4[:, 2] = -(tgtk[:, 0] + tgtk[:, 1])               # -c2
        tgt4[:, 3] = -2048.0                                  # sigmoid bias
        in_maps.append(
            {
                "iou": np.ascontiguousarray(pio.reshape(NCH * ML, CH)),
                "ab": np.ascontiguousarray(pab.reshape(NCH * ML, 2 * CH)),
                "cm": np.ascontiguousarray(pcm.reshape(NCH * ML, CH)),
                "tgt": tgt4,
            }
        )

    trace = bool(int(os.environ.get("BBK_TRACE", "0")))
    res = bass_utils.run_bass_kernel_spmd(
        nc, in_maps, core_ids=list(range(NCORES)), trace=trace
    )
    if trace:
        kernel.last_exec_time_ns = res.exec_time_ns

    acc = np.stack([res.results[k]["acc"] for k in range(NCORES)])  # [8,128,NCH]
    mm = np.stack(
        [
            res.results[k]["mm"].astype(np.float64)
            + res.results[k]["mmb"].astype(np.float64)
            for k in range(NCORES)
        ]
    )
    acc64 = acc.astype(np.float64)
    a_rows = acc64.sum(axis=2)                     # per-core per-row counts
    A = a_rows.sum()
    mm64 = mm.astype(np.float64)
    B1 = np.trace(mm64[:, :, 0:BLK], axis1=1, axis2=2).sum()
    B2 = np.trace(mm64[:, :, BLK : 2 * BLK], axis1=1, axis2=2).sum() / 4.0

    if a_rows.min() < 4 * TOPK:
        # threshold may not subsume top-3: replicate reference on host
        return _reference_numpy(
            out_moments, tgt_moments, num_targets, iou2ds, mask2d_np
        )

    return np.float32((A - B1 + B2) / A)


# revision 6
# speedup vs baseline: 1.1234x; 1.0114x over previous
"""Bass/Trainium2 kernel v3 for nn_BboxIoULoss (topk_masking).

loss = sum((1 - diou) * mask) / sum(mask),  mask = (iou1ds > 0.5) OR top-3
(top-3 subsumed by the threshold on these inputs; checked, numpy fallback
otherwise).

Strategy (8 cores data-parallel over M; per core 128 target partitions):
  - all four input slabs (iou, s1, e1, c1=s1+e1) ship as fp8 e4m3
    (4.2 MB/core): B1/B2 enter the loss scaled by ~0.019 so %-level error
    there moves the answer ~1e-3 << 2e-2 tolerance.
  - ACT: mask = Sigmoid(4096*iou - 2048) (hard 0/1 step at the fp8 grid
    point 0.5; values == 0.5 get weight 1/2, statistically neutral) with
    fused row-accumulate -> A; cd = c1 - c2 via Identity + per-partition
    bias.  A tiny warm-up activation hoists ACT_TABLE_LOAD to t~0.
  - DVE: two runtime-registered custom fused ops
      RW4_ANT:   rw  = relu(min(e1,e2) - max(s1,s2))        [4 stages]
      RENC8_ANT: renc ~= 1/(max(e1,e2) - min(s1,s2))        [8 stages]
        (enclose + bitwise-NOT reciprocal seed + 1 Newton step, ~0.4%)
    plus stock 2x-mode rm = mask*renc and qm = cd*rm.
  - TensorE: block-diagonal gram accumulation
      PB1 += rw_blk^T @ rm_blk   (trace = B1 = sum mask*iou)
      PB2 += qm_blk^T @ qm_blk   (trace = 4*B2 = 4*sum mask*pen)
  - answer = (A - B1 + B2) / A   (host sums traces in float64)
"""

import os
import ml_dtypes
import numpy as np

import concourse.bass as bass
import concourse.tile as tile
import concourse.mybir as mybir
from concourse import bacc, bass_utils

F32 = mybir.dt.float32
BF16 = mybir.dt.bfloat16
F8 = mybir.dt.float8e4
AF = mybir.ActivationFunctionType
OP = mybir.AluOpType

S = 64
T = 16
N = 128
M = S * T                  # 1024
P = N * (N + 1) // 2       # 8256
TOPK = 3
NCORES = 8
ML = M // NCORES           # 128 targets / core (= partitions)
W = S // NCORES            # 8 samples / core
NCH = int(os.environ.get("BBK2_NCH", "6"))
CH = P // NCH              # free-dim chunk
BLK = 128                  # matmul block width


def _patch_act_tables():
    """Pin one activation table-set (sigmoid/identity/copy) so the
    scheduler emits a single ACT_TABLE_LOAD."""
    import functools
    import concourse.hw_specs as _hw

    orig = _hw.get_activation_tables.__wrapped__

    def only_sigmoid(arch):
        tabs = orig(arch)
        name = "sigmoid_and_others"
        if name not in tabs:
            return tabs
        return {k: (v if k == name else set()) for k, v in tabs.items()}

    _hw.get_activation_tables = functools.cache(only_sigmoid)
    bacc.get_activation_tables = _hw.get_activation_tables


_OPS_REGISTERED = {}


def _register_dve_ops():
    """Register the two fused DVE ops at runtime (dve_ops.py is read-only)."""
    if _OPS_REGISTERED:
        return _OPS_REGISTERED
    import concourse.dve_ops as dve_ops
    from concourse.dve_spec import (
        Spec, Src0, Src1, C0, C1, C2, One, maxx, minn, lower, _has_src1,
        Bin, AluOp,
    )
    from concourse.dve_uop import (
        DveOpSpec, UopConfig, UopDpConfig, Trigger, InpSel, OutSel, OutPath,
        AluInp, DelayInp, ENABLE, AluOp as UAluOp,
    )
    from concourse.dve_table_gen import dve_ver_for

    ver = dve_ver_for("TRN2")

    def _rw4_2x_uop():
        """Hand-packed 2x_1P program for RW4: lo chain on blocks 0-3
        (result rides delay lane 0 to the end), hi chain on blocks 4-7
        (result in block 7's ALU flop).  Mirrors the stock tensor_tensor
        2x slot conventions (SRC_*_HI input lanes, write0_en_lo+hi)."""
        u = UopConfig()
        for lane, s in [
            (1, InpSel.SRC_1), (2, InpSel.CONST_1), (3, InpSel.SRC_0),
            (4, InpSel.CONST_0), (5, InpSel.SRC_0_HI), (6, InpSel.SRC_1_HI),
        ]:
            u.enable_input(s, lane)
        u.require_inp0 = ENABLE
        u.require_inp1 = ENABLE
        u.trigger = (Trigger.SRC_TENSOR_DONE, Trigger.NONE, Trigger.NONE)
        B = u.datapath_config
        # at block 0: PREV_DELAY_n = input lane n+1:
        #   D0=b_lo D1=e2 D2=a_lo D3=s2 D4=a_hi D5=b_hi
        B[0].enable_alu(UAluOp.MIN, AluInp.PREV_DELAY_0, AluInp.PREV_DELAY_1)
        B[0].pass_through_delay(1, 2, 3, 4, 5)          # v_lo in flop
        B[1].enable_alu(UAluOp.MAX, AluInp.PREV_DELAY_2, AluInp.PREV_DELAY_3)
        B[1].enable_delay_from_src(DelayInp.PREV_ALU_OUT, 0)   # D0 <- v_lo
        B[1].pass_through_delay(1, 3, 4, 5)             # u_lo in flop
        B[2].enable_alu(UAluOp.MAX, AluInp.PREV_DELAY_0, AluInp.PREV_ALU_OUT)
        B[2].enable_delay_from_src(DelayInp.PREV_ALU_OUT, 0)   # D0 <- u_lo
        B[2].pass_through_delay(1, 3, 4, 5)             # m_lo in flop
        B[3].enable_alu(UAluOp.SUBTRACT, AluInp.PREV_ALU_OUT, AluInp.PREV_DELAY_0)
        B[3].pass_through_delay(1, 3, 4, 5)             # rw_lo in flop
        B[4].enable_alu(UAluOp.MAX, AluInp.PREV_DELAY_4, AluInp.PREV_DELAY_3)
        B[4].enable_delay_from_src(DelayInp.PREV_ALU_OUT, 0)   # D0 <- rw_lo
        B[4].pass_through_delay(1, 5)                   # u_hi in flop
        B[5].enable_alu(UAluOp.MIN, AluInp.PREV_DELAY_5, AluInp.PREV_DELAY_1)
        B[5].enable_delay_from_src(DelayInp.PREV_ALU_OUT, 2)   # D2 <- u_hi
        B[5].pass_through_delay(0)                      # v_hi in flop
        B[6].enable_alu(UAluOp.MAX, AluInp.PREV_ALU_OUT, AluInp.PREV_DELAY_2)
        B[6].pass_through_delay(0, 2)                   # m_hi in flop
        B[7].enable_alu(UAluOp.SUBTRACT, AluInp.PREV_ALU_OUT, AluInp.PREV_DELAY_2)
        B[7].pass_through_delay(0)                      # rw_hi in flop
        u.enable_output(OutSel.DELAY_0, OutPath.WR0_LO)   # rw_lo
        u.enable_output(OutSel.ALU_OUT, OutPath.WR0_HI)   # rw_hi
        return u

    def _reg(name, spec):
        row = dve_ops._CUSTOM_DVE_ROW_BASE + len(dve_ops.OPS)
        lowered = DveOpSpec(
            name=name, opcode=row, uops=lower(spec, ver=ver),
            rd1_en=_has_src1(spec),
        )
        op = dve_ops.DveOp(
            name, spec, subdim=False, uops_sha={ver: lowered.sha(ver)}
        )
        dve_ops.OPS.append(op)
        dve_ops.CUSTOM_DVE_SPECS[name] = spec
        dve_ops._SUB_OPCODE_FOR_NAME[name] = row
        return op

    # rw = relu(min(b, e2) - max(a, s2)) = max(v, u) - u
    _u = maxx(Src0, C0)
    _v = minn(Src1, C1)
    _rw_body = maxx(_v, _u) - _u

    def _rw_ref(in0, in1, s0, s1, imm2):
        u = np.maximum(in0.astype(np.float32), s0)
        v = np.minimum(in1.astype(np.float32), s1)
        return np.maximum(v, u) - u

    def _reg2x(name, spec, uop2x_fn):
        class _PerfOp(dve_ops.DveOp):
            def compile(self, v):
                return DveOpSpec(
                    name=self.name,
                    opcode=dve_ops.get_dve_sub_opcode(self.name),
                    uops=lower(self.spec, ver=v),
                    rd1_en=_has_src1(self.spec),
                    uops_2x=[uop2x_fn()],
                    perf_max=1,
                )

        row = dve_ops._CUSTOM_DVE_ROW_BASE + len(dve_ops.OPS)
        op = _PerfOp(name, spec, subdim=False, uops_sha={})
        dve_ops.OPS.append(op)
        dve_ops.CUSTOM_DVE_SPECS[name] = spec
        dve_ops._SUB_OPCODE_FOR_NAME[name] = row
        return op

    rw4_spec = Spec(body=_rw_body, reference=_rw_ref)
    if os.environ.get("BBK2_RW2X", "1") == "1":
        rw4 = _reg2x("RW4_ANT", rw4_spec, _rw4_2x_uop)
    else:
        rw4 = _reg("RW4_ANT", rw4_spec)

    # nx = bitcast(~enc) ~= -4.25/enc, enc = max(b, e2) - min(a, s2).
    # The reciprocal seed scale (-4/17, so enc*(nx*c) lands within +-5.9%
    # of 1) is folded into the RM3 pass below; end-to-end loss error from
    # the seed-only reciprocal is ~1e-3 (measured), << 2e-2.
    _mn = minn(Src0, C0)
    _mx = maxx(Src1, C1)
    _enc = _mx - _mn
    _nx_body = Bin(AluOp.BITWISE_NOT, _enc, _enc)

    def _renc_ref(in0, in1, s0, s1, imm2):
        mn = np.minimum(in0.astype(np.float32), s0)
        mx = np.maximum(in1.astype(np.float32), s1)
        enc = (mx - mn).astype(np.float32)
        return (~enc.view(np.int32)).view(np.float32)

    def _renc4_2x_uop():
        u = UopConfig()
        for lane, s in [
            (1, InpSel.SRC_1), (2, InpSel.CONST_1), (3, InpSel.SRC_0),
            (4, InpSel.CONST_0), (5, InpSel.SRC_0_HI), (6, InpSel.SRC_1_HI),
        ]:
            u.enable_input(s, lane)
        u.require_inp0 = ENABLE
        u.require_inp1 = ENABLE
        u.trigger = (Trigger.SRC_TENSOR_DONE, Trigger.NONE, Trigger.NONE)
        B = u.datapath_config
        # block 0 PREV_DELAY_n = lane n+1: D0=b_lo D1=e2 D2=a_lo D3=s2
        #                                  D4=a_hi D5=b_hi
        B[0].enable_alu(UAluOp.MAX, AluInp.PREV_DELAY_0, AluInp.PREV_DELAY_1)
        B[0].pass_through_delay(1, 2, 3, 4, 5)          # mx_lo in flop
        B[1].enable_alu(UAluOp.MIN, AluInp.PREV_DELAY_2, AluInp.PREV_DELAY_3)
        B[1].enable_delay_from_src(DelayInp.PREV_ALU_OUT, 0)   # D0 <- mx_lo
        B[1].pass_through_delay(1, 3, 4, 5)             # mn_lo in flop
        B[2].enable_alu(UAluOp.SUBTRACT, AluInp.PREV_DELAY_0, AluInp.PREV_ALU_OUT)
        B[2].pass_through_delay(1, 3, 4, 5)             # enc_lo in flop
        B[3].enable_alu(UAluOp.BITWISE_NOT, AluInp.PREV_ALU_OUT, AluInp.PREV_ALU_OUT)
        B[3].pass_through_delay(1, 3, 4, 5)             # nx_lo in flop
        B[4].enable_alu(UAluOp.MAX, AluInp.PREV_DELAY_5, AluInp.PREV_DELAY_1)
        B[4].enable_delay_from_src(DelayInp.PREV_ALU_OUT, 0)   # D0 <- nx_lo
        B[4].pass_through_delay(3, 4)                   # mx_hi in flop
        B[5].enable_alu(UAluOp.MIN, AluInp.PREV_DELAY_4, AluInp.PREV_DELAY_3)
        B[5].enable_delay_from_src(DelayInp.PREV_ALU_OUT, 2)   # D2 <- mx_hi
        B[5].pass_through_delay(0)                      # mn_hi in flop
        B[6].enable_alu(UAluOp.SUBTRACT, AluInp.PREV_DELAY_2, AluInp.PREV_ALU_OUT)
        B[6].pass_through_delay(0)                      # enc_hi in flop
        B[7].enable_alu(UAluOp.BITWISE_NOT, AluInp.PREV_ALU_OUT, AluInp.PREV_ALU_OUT)
        B[7].pass_through_delay(0)                      # nx_hi in flop
        u.enable_output(OutSel.DELAY_0, OutPath.WR0_LO)   # nx_lo
        u.enable_output(OutSel.ALU_OUT, OutPath.WR0_HI)   # nx_hi
        return u

    renc4 = _reg2x(
        "RENC4_ANT", Spec(body=_nx_body, reference=_renc_ref), _renc4_2x_uop
    )

    # rm = mask * (nx * c0): the reciprocal-seed scale rides this pass
    _rm_body = (Src1 * C0) * Src0

    def _rm_ref(in0, in1, s0, s1, imm2):
        return (in1.astype(np.float32) * s0) * in0.astype(np.float32)

    def _rm3_2x_uop():
        u = UopConfig()
        for lane, s in [
            (1, InpSel.SRC_1), (2, InpSel.CONST_0), (3, InpSel.SRC_0),
            (4, InpSel.SRC_1_HI), (5, InpSel.SRC_0_HI),
        ]:
            u.enable_input(s, lane)
        u.require_inp0 = ENABLE
        u.require_inp1 = ENABLE
        u.trigger = (Trigger.SRC_TENSOR_DONE, Trigger.NONE, Trigger.NONE)
        B = u.datapath_config
        # block 0 PREV_DELAY_n = lane n+1: D0=nx_lo D1=c D2=mask_lo
        #                                  D3=nx_hi D4=mask_hi
        B[0].enable_alu(UAluOp.MULTIPLY, AluInp.PREV_DELAY_0, AluInp.PREV_DELAY_1)
        B[0].pass_through_delay(1, 2, 3, 4)             # t_lo in flop
        B[1].enable_alu(UAluOp.MULTIPLY, AluInp.PREV_ALU_OUT, AluInp.PREV_DELAY_2)
        B[1].pass_through_delay(1, 3, 4)                # rm_lo in flop
        B[2].enable_alu(UAluOp.MULTIPLY, AluInp.PREV_DELAY_3, AluInp.PREV_DELAY_1)
        B[2].enable_delay_from_src(DelayInp.PREV_ALU_OUT, 0)   # D0 <- rm_lo
        B[2].pass_through_delay(4)                      # t_hi in flop
        B[3].enable_alu(UAluOp.MULTIPLY, AluInp.PREV_ALU_OUT, AluInp.PREV_DELAY_4)
        B[3].pass_through_delay(0)                      # rm_hi in flop
        for k in (4, 5, 6, 7):
            B[k].pass_through_alu()                     # carry rm_hi
            B[k].pass_through_delay(0)                  # carry rm_lo
        u.enable_output(OutSel.DELAY_0, OutPath.WR0_LO)   # rm_lo
        u.enable_output(OutSel.ALU_OUT, OutPath.WR0_HI)   # rm_hi
        return u

    rm3 = _reg2x(
        "RM3_ANT", Spec(body=_rm_body, reference=_rm_ref), _rm3_2x_uop
    )

    _OPS_REGISTERED.update({"rw4": rw4, "renc4": renc4, "rm3": rm3})
    return _OPS_REGISTERED


def _build_program():
    if not os.environ.get("BBK2_NOPATCH"):
        _patch_act_tables()
    ops = _register_dve_ops()
    rw4, renc4, rm3 = ops["rw4"], ops["renc4"], ops["rm3"]

    nc = bacc.Bacc(
        "TRN2", target_bir_lowering=False, debug=False, enable_asserts=False
    )
    # chunk-major slabs: rows [c*ML, (c+1)*ML) = chunk c
    iou_d = nc.dram_tensor("iou", [NCH * ML, CH], F8, kind="ExternalInput")
    ab_d = nc.dram_tensor("ab", [NCH * ML, 2 * CH], BF16, kind="ExternalInput")
    cm_d = nc.dram_tensor("cm", [NCH * ML, CH], F8, kind="ExternalInput")
    tgt_d = nc.dram_tensor("tgt", [ML, 4], F32, kind="ExternalInput")
    acc_d = nc.dram_tensor("acc", [ML, NCH], F32, kind="ExternalOutput")
    mm_d = nc.dram_tensor("mm", [ML, 2 * BLK], F32, kind="ExternalOutput")
    mmb_d = nc.dram_tensor("mmb", [ML, 2 * BLK], F32, kind="ExternalOutput")

    linearize = bool(int(os.environ.get("BBK2_LINEARIZE", "0")))
    with tile.TileContext(nc, linearize=linearize) as tc:
        with (
            tc.tile_pool(name="const", bufs=1) as cp,
            tc.tile_pool(name="inp", bufs=int(os.environ.get("BBK2_IBUFS", "4"))) as ip,
            tc.tile_pool(name="work", bufs=int(os.environ.get("BBK2_WBUFS", "4"))) as wp,
            tc.psum_pool(name="ps", bufs=1) as pp,
        ):
            tgt = cp.tile([ML, 4], F32)
            s2 = tgt[:, 0:1]
            e2 = tgt[:, 1:2]
            nc2 = tgt[:, 2:3]          # -c2 (ACT Identity bias)
            sgb = tgt[:, 3:4]          # -2048 (sigmoid step bias)

            acc = cp.tile([ML, NCH], F32)
            pb1 = pp.tile([ML, BLK], F32)
            pb2 = pp.tile([ML, BLK], F32)
            pb1b = pp.tile([ML, BLK], F32)
            pb2b = pp.tile([ML, BLK], F32)

            # dummy activation: hoists ACT_TABLE_LOAD to t~0, concurrent
            # with the first input DMAs
            warm = cp.tile([ML, 1], F32)
            nc.vector.memset(warm[:], 0.0)
            nc.scalar.activation(warm[:], warm[:], AF.Sigmoid, bias=0.0, scale=1.0)

            nc.sync.dma_start(tgt[:], tgt_d.ap())

            nblk = (CH + BLK - 1) // BLK
            for c in range(NCH):
                abt = ip.tile([ML, 2 * CH], BF16, tag="abt")
                nc.sync.dma_start(abt[:], ab_d.ap()[c * ML : (c + 1) * ML, :])
                iot = ip.tile([ML, CH], F8, tag="iot")
                nc.sync.dma_start(iot[:], iou_d.ap()[c * ML : (c + 1) * ML, :])
                ct = ip.tile([ML, CH], F8, tag="ct")
                nc.sync.dma_start(ct[:], cm_d.ap()[c * ML : (c + 1) * ML, :])
                a = abt[:, 0:CH]           # s1
                b = abt[:, CH : 2 * CH]    # e1
                c1 = ct[:]

                # mask = hard step(iou > 0.5), fused row accumulate -> A
                mask = wp.tile([ML, CH], BF16, tag="mask")
                nc.scalar.activation(
                    mask[:], iot[:], AF.Sigmoid,
                    bias=sgb, scale=4096.0,
                    accum_out=acc[:, c : c + 1],
                )

                rw = wp.tile([ML, CH], BF16, tag="rw")
                _bi = nc.vector._custom_dve(
                    rw4, out=rw[:], in0=a, in1=b, s0=s2, s1=e2
                )
                for _attr in ("ins", "instruction", "inst"):
                    _obj = getattr(_bi, _attr, None)
                    if _obj is not None and hasattr(_obj, "perf_max"):
                        _obj.perf_max = 1
                        break
                else:
                    if hasattr(_bi, "perf_max"):
                        _bi.perf_max = 1
                renc = wp.tile([ML, CH], BF16, tag="renc")
                _b2 = nc.vector._custom_dve(
                    renc4, out=renc[:], in0=a, in1=b, s0=s2, s1=e2
                )
                rm = wp.tile([ML, CH], BF16, tag="rm")
                _b3 = nc.vector._custom_dve(
                    rm3, out=rm[:], in0=mask[:], in1=renc[:], s0=-4.0 / 17.0
                )
                for _o in (_b2, _b3):
                    for _attr in ("ins", "instruction", "inst"):
                        _obj = getattr(_o, _attr, None)
                        if _obj is not None and hasattr(_obj, "perf_max"):
                            _obj.perf_max = 1
                            break
                    else:
                        if hasattr(_o, "perf_max"):
                            _o.perf_max = 1

                cd = wp.tile([ML, CH], BF16, tag="cd")
                nc.scalar.activation(
                    cd[:], c1, AF.Identity, bias=nc2, scale=1.0
                )

                qm = wp.tile([ML, CH], BF16, tag="qm")
                nc.vector.tensor_tensor(qm[:], cd[:], rm[:], OP.mult)

                # chunks 0..NCH-2 accumulate into pb1/pb2; the last chunk
                # into pb1b/pb2b so the big accumulators drain (copy + DMA)
                # while the last chunk computes
                t1 = pb1 if c < NCH - 1 else pb1b
                t2 = pb2 if c < NCH - 1 else pb2b
                for bi in range(nblk):
                    lo = bi * BLK
                    hi = min(CH, lo + BLK)
                    wb = hi - lo
                    first = c in (0, NCH - 1) and bi == 0
                    last = c in (NCH - 2, NCH - 1) and bi == nblk - 1
                    nc.tensor.matmul(
                        t1[0:wb, 0:wb], rw[:, lo:hi], rm[:, lo:hi],
                        start=first, stop=last,
                    )
                    nc.tensor.matmul(
                        t2[0:wb, 0:wb], qm[:, lo:hi], qm[:, lo:hi],
                        start=first, stop=last,
                    )
                if c == NCH - 2:
                    mma = cp.tile([ML, 2 * BLK], F32)
                    nc.scalar.copy(mma[:, 0:BLK], pb1[:])
                    nc.scalar.copy(mma[:, BLK : 2 * BLK], pb2[:])
                    nc.sync.dma_start(mm_d.ap(), mma[:])

            mmb = cp.tile([ML, 2 * BLK], F32)
            nc.scalar.copy(mmb[:, 0:BLK], pb1b[:])
            nc.scalar.copy(mmb[:, BLK : 2 * BLK], pb2b[:])
            nc.sync.dma_start(mmb_d.ap(), mmb[:])
            nc.sync.dma_start(acc_d.ap(), acc[:])

    nc.compile()
    return nc


_NC_CACHE = None


def _get_program():
    global _NC_CACHE
    if _NC_CACHE is None:
        _NC_CACHE = _build_program()
    return _NC_CACHE


def _reference_numpy(out_moments, tgt_moments, num_targets, iou2ds, mask2d):
    """Exact numpy replica of the jax reference (fallback path)."""
    M_, N_, _ = iou2ds.shape
    S_, P_, _ = out_moments.shape
    scatter = np.repeat(np.arange(S_), num_targets)
    om = out_moments[scatter].astype(np.float32)      # [M, P, 2]
    tg = tgt_moments[:, None, :].astype(np.float32)
    s1, e1 = om[..., 0], om[..., 1]
    s2, e2 = tg[..., 0], tg[..., 1]
    inter = np.clip(np.minimum(e1, e2) - np.maximum(s1, s2), 0.0, None)
    union = (e1 - s1) + (e2 - s2) - inter
    iou = inter / union
    enclose = np.maximum(e1, e2) - np.minimum(s1, s2)
    cdist = (s1 + e1) * 0.5 - (s2 + e2) * 0.5
    bbox_diou = iou - (cdist * cdist) / (enclose * enclose)
    flat_idx = np.nonzero(mask2d.reshape(-1))[0]
    iou1 = iou2ds.reshape(M_, -1)[:, flat_idx]
    kth = np.argpartition(-iou1, TOPK - 1, axis=1)[:, :TOPK]
    target_mask = np.zeros((M_, P_), np.float32)
    target_mask[np.arange(M_)[:, None], kth] = 1.0
    target_mask = np.where(iou1 > 0.5, 1.0, target_mask)
    loss = 1.0 - bbox_diou
    return np.float32((loss * target_mask).sum() / target_mask.sum())


def kernel(out_moments, tgt_moments, num_targets, iou2ds, mask2d):
    out_moments = np.asarray(out_moments, np.float32)
    tgt_moments = np.asarray(tgt_moments, np.float32)
    num_targets = np.asarray(num_targets, np.int32)
    iou2ds = np.asarray(iou2ds, np.float32)
    mask2d_np = np.asarray(mask2d)

    uniform = bool(np.all(num_targets == T))
    triu_ok = bool(
        np.array_equal(mask2d_np, np.triu(np.ones((N, N), dtype=bool)))
    )
    if not (uniform and triu_ok and iou2ds.shape == (M, N, N)):
        return _reference_numpy(
            out_moments, tgt_moments, num_targets, iou2ds, mask2d_np
        )

    nc = _get_program()
    f8 = ml_dtypes.float8_e4m3

    # host layout prep: triu-compact iou2ds to p-order, fp8
    flat_idx = np.nonzero(mask2d_np.reshape(-1))[0]
    iou1 = iou2ds.reshape(M, -1)[:, flat_idx].astype(f8)     # [M, P]
    s1 = out_moments[..., 0]                                  # [S, P] f32
    e1 = out_moments[..., 1]
    bf16 = ml_dtypes.bfloat16
    c1 = (s1 + e1).astype(f8)
    s1 = s1.astype(bf16)
    e1 = e1.astype(bf16)

    in_maps = []
    for k in range(NCORES):
        sl_m = slice(k * ML, (k + 1) * ML)
        sl_s = slice(k * W, (k + 1) * W)
        # replicate each sample's moments across its 16 target partitions
        s1k = np.repeat(s1[sl_s], T, axis=0)                  # [128, P]
        e1k = np.repeat(e1[sl_s], T, axis=0)
        c1k = np.repeat(c1[sl_s], T, axis=0)
        iouk = iou1[sl_m]
        pio = np.empty((NCH, ML, CH), f8)
        pab = np.empty((NCH, ML, 2 * CH), bf16)
        pcm = np.empty((NCH, ML, CH), f8)
        for c in range(NCH):
            sl_p = slice(c * CH, (c + 1) * CH)
            pio[c] = iouk[:, sl_p]
            pab[c, :, 0:CH] = s1k[:, sl_p]
            pab[c, :, CH : 2 * CH] = e1k[:, sl_p]
            pcm[c] = c1k[:, sl_p]
        tgtk = tgt_moments[sl_m]                              # [128, 2] f32
        tgt4 = np.zeros((ML, 4), np.float32)
        tgt4[:, 0] = tgtk[:, 0]
        tgt4[:, 1] = tgtk[:, 1]
        tgt4[:, 2] = -(tgtk[:, 0] + tgtk[:, 1])               # -c2
        tgt4[:, 3] = -2048.0                                  # sigmoid bias
        in_maps.append(
            {
                "iou": np.ascontiguousarray(pio.reshape(NCH * ML, CH)),
                "ab": np.ascontiguousarray(pab.reshape(NCH * ML, 2 * CH)),
                "cm": np.ascontiguousarray(pcm.reshape(NCH * ML, CH)),
                "tgt": tgt4,
            }
        )

    trace = bool(int(os.environ.get("BBK_TRACE", "0")))
    res = bass_utils.run_bass_kernel_spmd(
        nc, in_maps, core_ids=list(range(NCORES)), trace=trace
    )
    if trace:
        kernel.last_exec_time_ns = res.exec_time_ns

    acc = np.stack([res.results[k]["acc"] for k in range(NCORES)])  # [8,128,NCH]
    mm = np.stack(
        [
            res.results[k]["mm"].astype(np.float64)
            + res.results[k]["mmb"].astype(np.float64)
            for k in range(NCORES)
        ]
    )
    acc64 = acc.astype(np.float64)
    a_rows = acc64.sum(axis=2)                     # per-core per-row counts
    A = a_rows.sum()
    mm64 = mm.astype(np.float64)
    B1 = np.trace(mm64[:, :, 0:BLK], axis1=1, axis2=2).sum()
    B2 = np.trace(mm64[:, :, BLK : 2 * BLK], axis1=1, axis2=2).sum() / 4.0

    if a_rows.min() < 4 * TOPK:
        # threshold may not subsume top-3: replicate reference on host
        return _reference_numpy(
            out_moments, tgt_moments, num_targets, iou2ds, mask2d_np
        )

    return np.float32((A - B1 + B2) / A)


# revision 7
# speedup vs baseline: 1.1260x; 1.0023x over previous
"""Bass/Trainium2 kernel v3 for nn_BboxIoULoss (topk_masking).

loss = sum((1 - diou) * mask) / sum(mask),  mask = (iou1ds > 0.5) OR top-3
(top-3 subsumed by the threshold on these inputs; checked, numpy fallback
otherwise).

Strategy (8 cores data-parallel over M; per core 128 target partitions):
  - all four input slabs (iou, s1, e1, c1=s1+e1) ship as fp8 e4m3
    (4.2 MB/core): B1/B2 enter the loss scaled by ~0.019 so %-level error
    there moves the answer ~1e-3 << 2e-2 tolerance.
  - ACT: mask = Sigmoid(4096*iou - 2048) (hard 0/1 step at the fp8 grid
    point 0.5; values == 0.5 get weight 1/2, statistically neutral) with
    fused row-accumulate -> A; cd = c1 - c2 via Identity + per-partition
    bias.  A tiny warm-up activation hoists ACT_TABLE_LOAD to t~0.
  - DVE: two runtime-registered custom fused ops
      RW4_ANT:   rw  = relu(min(e1,e2) - max(s1,s2))        [4 stages]
      RENC8_ANT: renc ~= 1/(max(e1,e2) - min(s1,s2))        [8 stages]
        (enclose + bitwise-NOT reciprocal seed + 1 Newton step, ~0.4%)
    plus stock 2x-mode rm = mask*renc and qm = cd*rm.
  - TensorE: block-diagonal gram accumulation
      PB1 += rw_blk^T @ rm_blk   (trace = B1 = sum mask*iou)
      PB2 += qm_blk^T @ qm_blk   (trace = 4*B2 = 4*sum mask*pen)
  - answer = (A - B1 + B2) / A   (host sums traces in float64)
"""

import os
import ml_dtypes
import numpy as np

import concourse.bass as bass
import concourse.tile as tile
import concourse.mybir as mybir
from concourse import bacc, bass_utils

F32 = mybir.dt.float32
BF16 = mybir.dt.bfloat16
F8 = mybir.dt.float8e4
AF = mybir.ActivationFunctionType
OP = mybir.AluOpType

S = 64
T = 16
N = 128
M = S * T                  # 1024
P = N * (N + 1) // 2       # 8256
TOPK = 3
NCORES = 8
ML = M // NCORES           # 128 targets / core (= partitions)
W = S // NCORES            # 8 samples / core
NCH = int(os.environ.get("BBK2_NCH", "6"))
CH = P // NCH              # free-dim chunk
BLK = 128                  # matmul block width


def _patch_act_tables():
    """Pin one activation table-set (sigmoid/identity/copy) so the
    scheduler emits a single ACT_TABLE_LOAD."""
    import functools
    import concourse.hw_specs as _hw

    orig = _hw.get_activation_tables.__wrapped__

    def only_sigmoid(arch):
        tabs = orig(arch)
        name = "sigmoid_and_others"
        if name not in tabs:
            return tabs
        return {k: (v if k == name else set()) for k, v in tabs.items()}

    _hw.get_activation_tables = functools.cache(only_sigmoid)
    bacc.get_activation_tables = _hw.get_activation_tables


_OPS_REGISTERED = {}


def _register_dve_ops():
    """Register the two fused DVE ops at runtime (dve_ops.py is read-only)."""
    if _OPS_REGISTERED:
        return _OPS_REGISTERED
    import concourse.dve_ops as dve_ops
    from concourse.dve_spec import (
        Spec, Src0, Src1, C0, C1, C2, One, maxx, minn, lower, _has_src1,
        Bin, AluOp,
    )
    from concourse.dve_uop import (
        DveOpSpec, UopConfig, UopDpConfig, Trigger, InpSel, OutSel, OutPath,
        AluInp, DelayInp, ENABLE, AluOp as UAluOp,
    )
    from concourse.dve_table_gen import dve_ver_for

    ver = dve_ver_for("TRN2")

    def _rw4_2x_uop():
        """Hand-packed 2x_1P program for RW4: lo chain on blocks 0-3
        (result rides delay lane 0 to the end), hi chain on blocks 4-7
        (result in block 7's ALU flop).  Mirrors the stock tensor_tensor
        2x slot conventions (SRC_*_HI input lanes, write0_en_lo+hi)."""
        u = UopConfig()
        for lane, s in [
            (1, InpSel.SRC_1), (2, InpSel.CONST_1), (3, InpSel.SRC_0),
            (4, InpSel.CONST_0), (5, InpSel.SRC_0_HI), (6, InpSel.SRC_1_HI),
        ]:
            u.enable_input(s, lane)
        u.require_inp0 = ENABLE
        u.require_inp1 = ENABLE
        u.trigger = (Trigger.SRC_TENSOR_DONE, Trigger.NONE, Trigger.NONE)
        B = u.datapath_config
        # at block 0: PREV_DELAY_n = input lane n+1:
        #   D0=b_lo D1=e2 D2=a_lo D3=s2 D4=a_hi D5=b_hi
        B[0].enable_alu(UAluOp.MIN, AluInp.PREV_DELAY_0, AluInp.PREV_DELAY_1)
        B[0].pass_through_delay(1, 2, 3, 4, 5)          # v_lo in flop
        B[1].enable_alu(UAluOp.MAX, AluInp.PREV_DELAY_2, AluInp.PREV_DELAY_3)
        B[1].enable_delay_from_src(DelayInp.PREV_ALU_OUT, 0)   # D0 <- v_lo
        B[1].pass_through_delay(1, 3, 4, 5)             # u_lo in flop
        B[2].enable_alu(UAluOp.MAX, AluInp.PREV_DELAY_0, AluInp.PREV_ALU_OUT)
        B[2].enable_delay_from_src(DelayInp.PREV_ALU_OUT, 0)   # D0 <- u_lo
        B[2].pass_through_delay(1, 3, 4, 5)             # m_lo in flop
        B[3].enable_alu(UAluOp.SUBTRACT, AluInp.PREV_ALU_OUT, AluInp.PREV_DELAY_0)
        B[3].pass_through_delay(1, 3, 4, 5)             # rw_lo in flop
        B[4].enable_alu(UAluOp.MAX, AluInp.PREV_DELAY_4, AluInp.PREV_DELAY_3)
        B[4].enable_delay_from_src(DelayInp.PREV_ALU_OUT, 0)   # D0 <- rw_lo
        B[4].pass_through_delay(1, 5)                   # u_hi in flop
        B[5].enable_alu(UAluOp.MIN, AluInp.PREV_DELAY_5, AluInp.PREV_DELAY_1)
        B[5].enable_delay_from_src(DelayInp.PREV_ALU_OUT, 2)   # D2 <- u_hi
        B[5].pass_through_delay(0)                      # v_hi in flop
        B[6].enable_alu(UAluOp.MAX, AluInp.PREV_ALU_OUT, AluInp.PREV_DELAY_2)
        B[6].pass_through_delay(0, 2)                   # m_hi in flop
        B[7].enable_alu(UAluOp.SUBTRACT, AluInp.PREV_ALU_OUT, AluInp.PREV_DELAY_2)
        B[7].pass_through_delay(0)                      # rw_hi in flop
        u.enable_output(OutSel.DELAY_0, OutPath.WR0_LO)   # rw_lo
        u.enable_output(OutSel.ALU_OUT, OutPath.WR0_HI)   # rw_hi
        return u

    def _reg(name, spec):
        row = dve_ops._CUSTOM_DVE_ROW_BASE + len(dve_ops.OPS)
        lowered = DveOpSpec(
            name=name, opcode=row, uops=lower(spec, ver=ver),
            rd1_en=_has_src1(spec),
        )
        op = dve_ops.DveOp(
            name, spec, subdim=False, uops_sha={ver: lowered.sha(ver)}
        )
        dve_ops.OPS.append(op)
        dve_ops.CUSTOM_DVE_SPECS[name] = spec
        dve_ops._SUB_OPCODE_FOR_NAME[name] = row
        return op

    # rw = relu(min(b, e2) - max(a, s2)) = max(v, u) - u
    _u = maxx(Src0, C0)
    _v = minn(Src1, C1)
    _rw_body = maxx(_v, _u) - _u

    def _rw_ref(in0, in1, s0, s1, imm2):
        u = np.maximum(in0.astype(np.float32), s0)
        v = np.minimum(in1.astype(np.float32), s1)
        return np.maximum(v, u) - u

    def _reg2x(name, spec, uop2x_fn):
        class _PerfOp(dve_ops.DveOp):
            def compile(self, v):
                return DveOpSpec(
                    name=self.name,
                    opcode=dve_ops.get_dve_sub_opcode(self.name),
                    uops=lower(self.spec, ver=v),
                    rd1_en=_has_src1(self.spec),
                    uops_2x=[uop2x_fn()],
                    perf_max=1,
                )

        row = dve_ops._CUSTOM_DVE_ROW_BASE + len(dve_ops.OPS)
        op = _PerfOp(name, spec, subdim=False, uops_sha={})
        dve_ops.OPS.append(op)
        dve_ops.CUSTOM_DVE_SPECS[name] = spec
        dve_ops._SUB_OPCODE_FOR_NAME[name] = row
        return op

    rw4_spec = Spec(body=_rw_body, reference=_rw_ref)
    if os.environ.get("BBK2_RW2X", "1") == "1":
        rw4 = _reg2x("RW4_ANT", rw4_spec, _rw4_2x_uop)
    else:
        rw4 = _reg("RW4_ANT", rw4_spec)

    # nx = bitcast(~enc) ~= -4.25/enc, enc = max(b, e2) - min(a, s2).
    # The reciprocal seed scale (-4/17, so enc*(nx*c) lands within +-5.9%
    # of 1) is folded into the RM3 pass below; end-to-end loss error from
    # the seed-only reciprocal is ~1e-3 (measured), << 2e-2.
    _mn = minn(Src0, C0)
    _mx = maxx(Src1, C1)
    _enc = _mx - _mn
    _nx_body = Bin(AluOp.BITWISE_NOT, _enc, _enc)

    def _renc_ref(in0, in1, s0, s1, imm2):
        mn = np.minimum(in0.astype(np.float32), s0)
        mx = np.maximum(in1.astype(np.float32), s1)
        enc = (mx - mn).astype(np.float32)
        return (~enc.view(np.int32)).view(np.float32)

    def _renc4_2x_uop():
        u = UopConfig()
        for lane, s in [
            (1, InpSel.SRC_1), (2, InpSel.CONST_1), (3, InpSel.SRC_0),
            (4, InpSel.CONST_0), (5, InpSel.SRC_0_HI), (6, InpSel.SRC_1_HI),
        ]:
            u.enable_input(s, lane)
        u.require_inp0 = ENABLE
        u.require_inp1 = ENABLE
        u.trigger = (Trigger.SRC_TENSOR_DONE, Trigger.NONE, Trigger.NONE)
        B = u.datapath_config
        # block 0 PREV_DELAY_n = lane n+1: D0=b_lo D1=e2 D2=a_lo D3=s2
        #                                  D4=a_hi D5=b_hi
        B[0].enable_alu(UAluOp.MAX, AluInp.PREV_DELAY_0, AluInp.PREV_DELAY_1)
        B[0].pass_through_delay(1, 2, 3, 4, 5)          # mx_lo in flop
        B[1].enable_alu(UAluOp.MIN, AluInp.PREV_DELAY_2, AluInp.PREV_DELAY_3)
        B[1].enable_delay_from_src(DelayInp.PREV_ALU_OUT, 0)   # D0 <- mx_lo
        B[1].pass_through_delay(1, 3, 4, 5)             # mn_lo in flop
        B[2].enable_alu(UAluOp.SUBTRACT, AluInp.PREV_DELAY_0, AluInp.PREV_ALU_OUT)
        B[2].pass_through_delay(1, 3, 4, 5)             # enc_lo in flop
        B[3].enable_alu(UAluOp.BITWISE_NOT, AluInp.PREV_ALU_OUT, AluInp.PREV_ALU_OUT)
        B[3].pass_through_delay(1, 3, 4, 5)             # nx_lo in flop
        B[4].enable_alu(UAluOp.MAX, AluInp.PREV_DELAY_5, AluInp.PREV_DELAY_1)
        B[4].enable_delay_from_src(DelayInp.PREV_ALU_OUT, 0)   # D0 <- nx_lo
        B[4].pass_through_delay(3, 4)                   # mx_hi in flop
        B[5].enable_alu(UAluOp.MIN, AluInp.PREV_DELAY_4, AluInp.PREV_DELAY_3)
        B[5].enable_delay_from_src(DelayInp.PREV_ALU_OUT, 2)   # D2 <- mx_hi
        B[5].pass_through_delay(0)                      # mn_hi in flop
        B[6].enable_alu(UAluOp.SUBTRACT, AluInp.PREV_DELAY_2, AluInp.PREV_ALU_OUT)
        B[6].pass_through_delay(0)                      # enc_hi in flop
        B[7].enable_alu(UAluOp.BITWISE_NOT, AluInp.PREV_ALU_OUT, AluInp.PREV_ALU_OUT)
        B[7].pass_through_delay(0)                      # nx_hi in flop
        u.enable_output(OutSel.DELAY_0, OutPath.WR0_LO)   # nx_lo
        u.enable_output(OutSel.ALU_OUT, OutPath.WR0_HI)   # nx_hi
        return u

    renc4 = _reg2x(
        "RENC4_ANT", Spec(body=_nx_body, reference=_renc_ref), _renc4_2x_uop
    )

    # rm = mask * (nx * c0): the reciprocal-seed scale rides this pass
    _rm_body = (Src1 * C0) * Src0

    def _rm_ref(in0, in1, s0, s1, imm2):
        return (in1.astype(np.float32) * s0) * in0.astype(np.float32)

    def _rm3_2x_uop():
        u = UopConfig()
        for lane, s in [
            (1, InpSel.SRC_1), (2, InpSel.CONST_0), (3, InpSel.SRC_0),
            (4, InpSel.SRC_1_HI), (5, InpSel.SRC_0_HI),
        ]:
            u.enable_input(s, lane)
        u.require_inp0 = ENABLE
        u.require_inp1 = ENABLE
        u.trigger = (Trigger.SRC_TENSOR_DONE, Trigger.NONE, Trigger.NONE)
        B = u.datapath_config
        # block 0 PREV_DELAY_n = lane n+1: D0=nx_lo D1=c D2=mask_lo
        #                                  D3=nx_hi D4=mask_hi
        B[0].enable_alu(UAluOp.MULTIPLY, AluInp.PREV_DELAY_0, AluInp.PREV_DELAY_1)
        B[0].pass_through_delay(1, 2, 3, 4)             # t_lo in flop
        B[1].enable_alu(UAluOp.MULTIPLY, AluInp.PREV_ALU_OUT, AluInp.PREV_DELAY_2)
        B[1].pass_through_delay(1, 3, 4)                # rm_lo in flop
        B[2].enable_alu(UAluOp.MULTIPLY, AluInp.PREV_DELAY_3, AluInp.PREV_DELAY_1)
        B[2].enable_delay_from_src(DelayInp.PREV_ALU_OUT, 0)   # D0 <- rm_lo
        B[2].pass_through_delay(4)                      # t_hi in flop
        B[3].enable_alu(UAluOp.MULTIPLY, AluInp.PREV_ALU_OUT, AluInp.PREV_DELAY_4)
        B[3].pass_through_delay(0)                      # rm_hi in flop
        for k in (4, 5, 6, 7):
            B[k].pass_through_alu()                     # carry rm_hi
            B[k].pass_through_delay(0)                  # carry rm_lo
        u.enable_output(OutSel.DELAY_0, OutPath.WR0_LO)   # rm_lo
        u.enable_output(OutSel.ALU_OUT, OutPath.WR0_HI)   # rm_hi
        return u

    rm3 = _reg2x(
        "RM3_ANT", Spec(body=_rm_body, reference=_rm_ref), _rm3_2x_uop
    )

    _OPS_REGISTERED.update({"rw4": rw4, "renc4": renc4, "rm3": rm3})
    return _OPS_REGISTERED


def _build_program():
    if not os.environ.get("BBK2_NOPATCH"):
        _patch_act_tables()
    ops = _register_dve_ops()
    rw4, renc4, rm3 = ops["rw4"], ops["renc4"], ops["rm3"]

    nc = bacc.Bacc(
        "TRN2", target_bir_lowering=False, debug=False, enable_asserts=False
    )
    # chunk-major slabs: rows [c*ML, (c+1)*ML) = chunk c
    iou_d = nc.dram_tensor("iou", [NCH * ML, CH], F8, kind="ExternalInput")
    ab_d = nc.dram_tensor("ab", [NCH * ML, 2 * CH], BF16, kind="ExternalInput")
    cm_d = nc.dram_tensor("cm", [NCH * ML, CH], F8, kind="ExternalInput")
    tgt_d = nc.dram_tensor("tgt", [ML, 4], F32, kind="ExternalInput")
    acc_d = nc.dram_tensor("acc", [ML, NCH], F32, kind="ExternalOutput")
    mm_d = nc.dram_tensor("mm", [ML, 2 * BLK], BF16, kind="ExternalOutput")
    mmb_d = nc.dram_tensor("mmb", [ML, 2 * BLK], BF16, kind="ExternalOutput")

    linearize = bool(int(os.environ.get("BBK2_LINEARIZE", "0")))
    with tile.TileContext(nc, linearize=linearize) as tc:
        with (
            tc.tile_pool(name="const", bufs=1) as cp,
            tc.tile_pool(name="inp", bufs=int(os.environ.get("BBK2_IBUFS", "4"))) as ip,
            tc.tile_pool(name="work", bufs=int(os.environ.get("BBK2_WBUFS", "4"))) as wp,
            tc.psum_pool(name="ps", bufs=1) as pp,
        ):
            tgt = cp.tile([ML, 4], F32)
            s2 = tgt[:, 0:1]
            e2 = tgt[:, 1:2]
            nc2 = tgt[:, 2:3]          # -c2 (ACT Identity bias)
            sgb = tgt[:, 3:4]          # -2048 (sigmoid step bias)

            acc = cp.tile([ML, NCH], F32)
            pb1 = pp.tile([ML, BLK], F32)
            pb2 = pp.tile([ML, BLK], F32)
            pb1b = pp.tile([ML, BLK], F32)
            pb2b = pp.tile([ML, BLK], F32)

            # dummy activation: hoists ACT_TABLE_LOAD to t~0, concurrent
            # with the first input DMAs
            warm = cp.tile([ML, 1], F32)
            nc.vector.memset(warm[:], 0.0)
            nc.scalar.activation(warm[:], warm[:], AF.Sigmoid, bias=0.0, scale=1.0)

            nc.sync.dma_start(tgt[:], tgt_d.ap())

            nblk = (CH + BLK - 1) // BLK
            for c in range(NCH):
                abt = ip.tile([ML, 2 * CH], BF16, tag="abt")
                nc.sync.dma_start(abt[:], ab_d.ap()[c * ML : (c + 1) * ML, :])
                iot = ip.tile([ML, CH], F8, tag="iot")
                nc.sync.dma_start(iot[:], iou_d.ap()[c * ML : (c + 1) * ML, :])
                ct = ip.tile([ML, CH], F8, tag="ct")
                nc.sync.dma_start(ct[:], cm_d.ap()[c * ML : (c + 1) * ML, :])
                a = abt[:, 0:CH]           # s1
                b = abt[:, CH : 2 * CH]    # e1
                c1 = ct[:]

                # mask = hard step(iou > 0.5), fused row accumulate -> A
                mask = wp.tile([ML, CH], BF16, tag="mask")
                nc.scalar.activation(
                    mask[:], iot[:], AF.Sigmoid,
                    bias=sgb, scale=4096.0,
                    accum_out=acc[:, c : c + 1],
                )

                rw = wp.tile([ML, CH], BF16, tag="rw")
                _bi = nc.vector._custom_dve(
                    rw4, out=rw[:], in0=a, in1=b, s0=s2, s1=e2
                )
                for _attr in ("ins", "instruction", "inst"):
                    _obj = getattr(_bi, _attr, None)
                    if _obj is not None and hasattr(_obj, "perf_max"):
                        _obj.perf_max = 1
                        break
                else:
                    if hasattr(_bi, "perf_max"):
                        _bi.perf_max = 1
                renc = wp.tile([ML, CH], BF16, tag="renc")
                _b2 = nc.vector._custom_dve(
                    renc4, out=renc[:], in0=a, in1=b, s0=s2, s1=e2
                )
                rm = wp.tile([ML, CH], BF16, tag="rm")
                _b3 = nc.vector._custom_dve(
                    rm3, out=rm[:], in0=mask[:], in1=renc[:], s0=-4.0 / 17.0
                )
                for _o in (_b2, _b3):
                    for _attr in ("ins", "instruction", "inst"):
                        _obj = getattr(_o, _attr, None)
                        if _obj is not None and hasattr(_obj, "perf_max"):
                            _obj.perf_max = 1
                            break
                    else:
                        if hasattr(_o, "perf_max"):
                            _o.perf_max = 1

                cd = wp.tile([ML, CH], BF16, tag="cd")
                nc.scalar.activation(
                    cd[:], c1, AF.Identity, bias=nc2, scale=1.0
                )

                qm = wp.tile([ML, CH], BF16, tag="qm")
                nc.vector.tensor_tensor(qm[:], cd[:], rm[:], OP.mult)

                # chunks 0..NCH-2 accumulate into pb1/pb2; the last chunk
                # into pb1b/pb2b so the big accumulators drain (copy + DMA)
                # while the last chunk computes
                t1 = pb1 if c < NCH - 1 else pb1b
                t2 = pb2 if c < NCH - 1 else pb2b
                for bi in range(nblk):
                    lo = bi * BLK
                    hi = min(CH, lo + BLK)
                    wb = hi - lo
                    first = c in (0, NCH - 1) and bi == 0
                    last = c in (NCH - 2, NCH - 1) and bi == nblk - 1
                    nc.tensor.matmul(
                        t1[0:wb, 0:wb], rw[:, lo:hi], rm[:, lo:hi],
                        start=first, stop=last,
                    )
                    nc.tensor.matmul(
                        t2[0:wb, 0:wb], qm[:, lo:hi], qm[:, lo:hi],
                        start=first, stop=last,
                    )
                if c == NCH - 2:
                    mma = cp.tile([ML, 2 * BLK], BF16)
                    nc.scalar.copy(mma[:, 0:BLK], pb1[:])
                    nc.scalar.copy(mma[:, BLK : 2 * BLK], pb2[:])
                    nc.sync.dma_start(mm_d.ap(), mma[:])

            mmb = cp.tile([ML, 2 * BLK], BF16)
            nc.scalar.copy(mmb[:, 0:BLK], pb1b[:])
            nc.scalar.copy(mmb[:, BLK : 2 * BLK], pb2b[:])
            nc.sync.dma_start(mmb_d.ap(), mmb[:])
            nc.sync.dma_start(acc_d.ap(), acc[:])

    nc.compile()
    return nc


_NC_CACHE = None


def _get_program():
    global _NC_CACHE
    if _NC_CACHE is None:
        _NC_CACHE = _build_program()
    return _NC_CACHE


def _reference_numpy(out_moments, tgt_moments, num_targets, iou2ds, mask2d):
    """Exact numpy replica of the jax reference (fallback path)."""
    M_, N_, _ = iou2ds.shape
    S_, P_, _ = out_moments.shape
    scatter = np.repeat(np.arange(S_), num_targets)
    om = out_moments[scatter].astype(np.float32)      # [M, P, 2]
    tg = tgt_moments[:, None, :].astype(np.float32)
    s1, e1 = om[..., 0], om[..., 1]
    s2, e2 = tg[..., 0], tg[..., 1]
    inter = np.clip(np.minimum(e1, e2) - np.maximum(s1, s2), 0.0, None)
    union = (e1 - s1) + (e2 - s2) - inter
    iou = inter / union
    enclose = np.maximum(e1, e2) - np.minimum(s1, s2)
    cdist = (s1 + e1) * 0.5 - (s2 + e2) * 0.5
    bbox_diou = iou - (cdist * cdist) / (enclose * enclose)
    flat_idx = np.nonzero(mask2d.reshape(-1))[0]
    iou1 = iou2ds.reshape(M_, -1)[:, flat_idx]
    kth = np.argpartition(-iou1, TOPK - 1, axis=1)[:, :TOPK]
    target_mask = np.zeros((M_, P_), np.float32)
    target_mask[np.arange(M_)[:, None], kth] = 1.0
    target_mask = np.where(iou1 > 0.5, 1.0, target_mask)
    loss = 1.0 - bbox_diou
    return np.float32((loss * target_mask).sum() / target_mask.sum())


def kernel(out_moments, tgt_moments, num_targets, iou2ds, mask2d):
    out_moments = np.asarray(out_moments, np.float32)
    tgt_moments = np.asarray(tgt_moments, np.float32)
    num_targets = np.asarray(num_targets, np.int32)
    iou2ds = np.asarray(iou2ds, np.float32)
    mask2d_np = np.asarray(mask2d)

    uniform = bool(np.all(num_targets == T))
    triu_ok = bool(
        np.array_equal(mask2d_np, np.triu(np.ones((N, N), dtype=bool)))
    )
    if not (uniform and triu_ok and iou2ds.shape == (M, N, N)):
        return _reference_numpy(
            out_moments, tgt_moments, num_targets, iou2ds, mask2d_np
        )

    nc = _get_program()
    f8 = ml_dtypes.float8_e4m3

    # host layout prep: triu-compact iou2ds to p-order, fp8
    flat_idx = np.nonzero(mask2d_np.reshape(-1))[0]
    iou1 = iou2ds.reshape(M, -1)[:, flat_idx].astype(f8)     # [M, P]
    s1 = out_moments[..., 0]                                  # [S, P] f32
    e1 = out_moments[..., 1]
    bf16 = ml_dtypes.bfloat16
    c1 = (s1 + e1).astype(f8)
    s1 = s1.astype(bf16)
    e1 = e1.astype(bf16)

    in_maps = []
    for k in range(NCORES):
        sl_m = slice(k * ML, (k + 1) * ML)
        sl_s = slice(k * W, (k + 1) * W)
        # replicate each sample's moments across its 16 target partitions
        s1k = np.repeat(s1[sl_s], T, axis=0)                  # [128, P]
        e1k = np.repeat(e1[sl_s], T, axis=0)
        c1k = np.repeat(c1[sl_s], T, axis=0)
        iouk = iou1[sl_m]
        pio = np.empty((NCH, ML, CH), f8)
        pab = np.empty((NCH, ML, 2 * CH), bf16)
        pcm = np.empty((NCH, ML, CH), f8)
        for c in range(NCH):
            sl_p = slice(c * CH, (c + 1) * CH)
            pio[c] = iouk[:, sl_p]
            pab[c, :, 0:CH] = s1k[:, sl_p]
            pab[c, :, CH : 2 * CH] = e1k[:, sl_p]
            pcm[c] = c1k[:, sl_p]
        tgtk = tgt_moments[sl_m]                              # [128, 2] f32
        tgt4 = np.zeros((ML, 4), np.float32)
        tgt4[:, 0] = tgtk[:, 0]
        tgt4[:, 1] = tgtk[:, 1]
        tgt4[:, 2] = -(tgtk[:, 0] + tgtk[:, 1])               # -c2
        tgt4[:, 3] = -2048.0                                  # sigmoid bias
        in_maps.append(
            {
                "iou": np.ascontiguousarray(pio.reshape(NCH * ML, CH)),
                "ab": np.ascontiguousarray(pab.reshape(NCH * ML, 2 * CH)),
                "cm": np.ascontiguousarray(pcm.reshape(NCH * ML, CH)),
                "tgt": tgt4,
            }
        )

    trace = bool(int(os.environ.get("BBK_TRACE", "0")))
    res = bass_utils.run_bass_kernel_spmd(
        nc, in_maps, core_ids=list(range(NCORES)), trace=trace
    )
    if trace:
        kernel.last_exec_time_ns = res.exec_time_ns

    acc = np.stack([res.results[k]["acc"] for k in range(NCORES)])  # [8,128,NCH]
    mm = np.stack(
        [
            res.results[k]["mm"].astype(np.float64)
            + res.results[k]["mmb"].astype(np.float64)
            for k in range(NCORES)
        ]
    )
    acc64 = acc.astype(np.float64)
    a_rows = acc64.sum(axis=2)                     # per-core per-row counts
    A = a_rows.sum()
    mm64 = mm.astype(np.float64)
    B1 = np.trace(mm64[:, :, 0:BLK], axis1=1, axis2=2).sum()
    B2 = np.trace(mm64[:, :, BLK : 2 * BLK], axis1=1, axis2=2).sum() / 4.0

    if a_rows.min() < 4 * TOPK:
        # threshold may not subsume top-3: replicate reference on host
        return _reference_numpy(
            out_moments, tgt_moments, num_targets, iou2ds, mask2d_np
        )

    return np.float32((A - B1 + B2) / A)
